# revision 1
# baseline (speedup 1.0000x reference)
"""Causal self-attention (B=2, S=2048, D=2048, H=16) on 8 TRN2 NeuronCores.

Sharding: tensor-parallel over heads x data-parallel over batch.
Core c = b*4 + g handles batch b and heads 4g..4g+3 (head_dim=128).

Per-core device kernel (single NEFF, SPMD across 8 cores):
  stage 1: q/k projections into transposed layout qT/kT [d, S] (f32r matmuls),
           v projection into natural layout [S, d] stored bf16 with a fused
           ones-column per head (for softmax denominators).
  stage 2: per (head, 512-wide q-block): scores in transposed layout
           sT[j] = kT_j.T @ qT  (f32r, PSUM fp32); probs = exp(sT) -> bf16;
           diagonal-strip blocks multiplied by a static binary causal mask;
           av[q,128+1] = probs_j.T @ [v_j | 1] accumulated over j (bf16);
           row-normalize by reciprocal of the ones-column; PE-transpose to
           attoutT [d, q] (f32r).
  stage 3: y_partial = attoutT.T @ Wo_shard (f32r), DMA out fp32.

Softmax skips the max-subtraction (scores are O(5) for the expected input
distribution; a host-side sampling guard falls back to a numpy reference if
scores could overflow exp, or if the mask is not the canonical causal mask).

Host: y[b] = sum of the 4 per-core partials for that batch.
"""

import math
from contextlib import ExitStack

import numpy as np

B = 2
S = 2048
D = 2048
H = 16
HPC = 4  # heads per core
d = 128  # head dim
N_CORES = 8
P = 128
DK = D // P  # 16 contraction tiles
ST = S // P  # 16 token tiles
QB = S // 512  # 4 q-blocks of 512

_CACHE = {}


def _build_module(repeat=1):
    import concourse.mybir as mybir
    import concourse.tile as tile
    from concourse import bacc

    f32 = mybir.dt.float32
    f32r = mybir.dt.float32r
    bf16 = mybir.dt.bfloat16
    Exp = mybir.ActivationFunctionType.Exp

    nc = bacc.Bacc("TRN2", target_bir_lowering=False, debug=False)

    xT = nc.dram_tensor("xT", [D, S], f32r, kind="ExternalInput")
    wq = nc.dram_tensor("wq", [D, HPC * d], f32r, kind="ExternalInput")
    wk = nc.dram_tensor("wk", [D, HPC * d], f32r, kind="ExternalInput")
    wv = nc.dram_tensor("wv", [D, HPC * d], f32r, kind="ExternalInput")
    wo = nc.dram_tensor("wo", [HPC * d, D], f32r, kind="ExternalInput")
    bm = nc.dram_tensor("bm", [4, P, 512], bf16, kind="ExternalInput")
    ident = nc.dram_tensor("ident", [P, P], f32r, kind="ExternalInput")
    y = nc.dram_tensor("y", [S, D], f32, kind="ExternalOutput")

    xT_r = xT.ap().rearrange("(t p) s -> p t s", p=P)
    y_r = y.ap().rearrange("(t p) n -> p t n", p=P)

    with tile.TileContext(nc) as tc, ExitStack() as top:
        # pools that live across stages
        qkp = top.enter_context(tc.tile_pool(name="qkp", bufs=1))
        vp = top.enter_context(tc.tile_pool(name="vp", bufs=1))
        mp = top.enter_context(tc.tile_pool(name="mp", bufs=1))

        qT_sb = qkp.tile([P, HPC, S], f32r, tag="qT")
        kT_sb = qkp.tile([P, HPC, S], f32r, tag="kT")
        v_sb = vp.tile([P, ST, HPC, d + 1], bf16, tag="v")
        mask_sb = mp.tile([P, 4, 512], bf16, tag="bm")
        id_sb = mp.tile([P, P], f32r, tag="ident")

        nc.vector.memset(v_sb[:, :, :, d : d + 1], 1.0)

        for _rep in range(repeat):
            wv_r = wv.ap().rearrange("(t p) m -> p t m", p=P)
            wv_pc = []
            s1x = ExitStack()
            wvpA = s1x.enter_context(tc.tile_pool(name="wvpA", bufs=1))

            # ---- stage 1a: q/k projections (tok chunks of 256) ----
            with ExitStack() as s1a:
                wqk = s1a.enter_context(tc.tile_pool(name="wqk", bufs=1))
                xap = s1a.enter_context(tc.tile_pool(name="xap", bufs=2))
                psa = s1a.enter_context(tc.tile_pool(name="psa", bufs=3, space="PSUM"))

                wq_r = wq.ap().rearrange("(t p) m -> p t m", p=P)
                wk_r = wk.ap().rearrange("(t p) m -> p t m", p=P)
                wq_pc, wk_pc = [], []
                for piece in range(4):
                    kk0, kk1 = piece * 4, (piece + 1) * 4
                    tq = wqk.tile([P, 4, HPC * d], f32r, tag=f"wq{piece}")
                    tk = wqk.tile([P, 4, HPC * d], f32r, tag=f"wk{piece}")
                    nc.sync.dma_start(out=tq, in_=wq_r[:, kk0:kk1, :])
                    wq_pc.append(tq)
                    wk_pc.append(tk)
                # prefetch first half of wv behind wq on the SP ring so
                # stage 1b can start without waiting for stage-1a space release
                for piece in range(2):
                    kk0, kk1 = piece * 4, (piece + 1) * 4
                    t = wvpA.tile([P, 4, HPC * d], f32r, tag=f"wv{piece}")
                    nc.sync.dma_start(out=t, in_=wv_r[:, kk0:kk1, :])
                    wv_pc.append(t)
                xb0 = wvpA.tile([P, 4, 512], f32r, tag="xb0pre")
                nc.sync.dma_start(out=xb0, in_=xT_r[:, 0:4, 0:512])

                for c in range(S // 256):
                    xa_pc = []
                    for piece in range(4):
                        kk0, kk1 = piece * 4, (piece + 1) * 4
                        t = xap.tile([P, 4, 256], f32r, tag=f"xa{piece}")
                        nc.scalar.dma_start(
                            out=t, in_=xT_r[:, kk0:kk1, c * 256 : (c + 1) * 256]
                        )
                        xa_pc.append(t)
                    if c == 0:
                        # wk rides the ACT ring right behind chunk-0 x pieces,
                        # in parallel with wq on the SP ring
                        for piece in range(4):
                            kk0, kk1 = piece * 4, (piece + 1) * 4
                            nc.scalar.dma_start(
                                out=wk_pc[piece], in_=wk_r[:, kk0:kk1, :]
                            )
                    for w_pc, dest in ((wq_pc, qT_sb), (wk_pc, kT_sb)):
                        for m in range(HPC):
                            ps = psa.tile([P, 256], f32, tag="pa")
                            for kk in range(DK):
                                nc.tensor.matmul(
                                    ps,
                                    w_pc[kk // 4][:, kk % 4, m * d : (m + 1) * d],
                                    xa_pc[kk // 4][:, kk % 4, :],
                                    start=(kk == 0),
                                    stop=(kk == DK - 1),
                                )
                            nc.vector.tensor_copy(
                                dest[:, m, c * 256 : (c + 1) * 256], ps
                            )

            # ---- stage 1b: v projection (tok chunks of 512) ----
            with ExitStack() as s1b:
                wvp = s1b.enter_context(tc.tile_pool(name="wvp", bufs=1))
                xbp = s1b.enter_context(tc.tile_pool(name="xbp", bufs=2))
                psb = s1b.enter_context(tc.tile_pool(name="psb", bufs=3, space="PSUM"))

                for piece in range(2, 4):
                    kk0, kk1 = piece * 4, (piece + 1) * 4
                    t = wvp.tile([P, 4, HPC * d], f32r, tag=f"wv{piece}")
                    nc.sync.dma_start(out=t, in_=wv_r[:, kk0:kk1, :])
                    wv_pc.append(t)

                for c in range(S // 512):
                    xb_pc = []
                    for piece in range(4):
                        if c == 0 and piece == 0:
                            xb_pc.append(xb0)
                            continue
                        kk0, kk1 = piece * 4, (piece + 1) * 4
                        t = xbp.tile([P, 4, 512], f32r, tag=f"xb{piece}")
                        nc.scalar.dma_start(
                            out=t, in_=xT_r[:, kk0:kk1, c * 512 : (c + 1) * 512]
                        )
                        xb_pc.append(t)
                    for mt in range(4):
                        ps = psb.tile([P, 512], f32, tag="pb")
                        for kk in range(DK):
                            nc.tensor.matmul(
                                ps,
                                xb_pc[kk // 4][:, kk % 4, mt * P : (mt + 1) * P],
                                wv_pc[kk // 4][:, kk % 4, :],
                                start=(kk == 0),
                                stop=(kk == DK - 1),
                            )
                        ti = c * 4 + mt
                        nc.vector.tensor_copy(
                            v_sb[:, ti, :, 0:d],
                            ps.rearrange("p (h e) -> p h e", h=HPC),
                        )

            s1x.close()
            # pools for stages 2-3 open only after stage-1 pools are released
            s23 = ExitStack()
            aop = s23.enter_context(tc.tile_pool(name="aop", bufs=1))
            wop = s23.enter_context(tc.tile_pool(name="wop", bufs=1))
            attoutT_sb = aop.tile([P, HPC, S], f32r, tag="attoutT")
            wo_sb = wop.tile([P, HPC, D], f32r, tag="wo")
            # load wo + stage-2 constants while attention runs
            nc.sync.dma_start(out=wo_sb, in_=wo.ap().rearrange("(t p) n -> p t n", p=P))
            if _rep == 0:
                nc.scalar.dma_start(
                    out=mask_sb, in_=bm.ap().rearrange("r p m -> p r m")
                )
                nc.scalar.dma_start(out=id_sb, in_=ident.ap())

            # shared by stage-2 scores and stage-3 out-proj (overlap enabler)
            ps_sc = s23.enter_context(tc.tile_pool(name="ps_sc", bufs=3, space="PSUM"))

            # ---- stage 2: attention ----
            with ExitStack() as s2:
                probp = s2.enter_context(tc.tile_pool(name="probp", bufs=24))
                smallp = s2.enter_context(tc.tile_pool(name="smallp", bufs=3))
                ps_av = s2.enter_context(tc.tile_pool(name="ps_av", bufs=3, space="PSUM"))
                ps_tr = s2.enter_context(tc.tile_pool(name="ps_tr", bufs=2, space="PSUM"))

                for h in range(HPC):
                    for Q in range(QB):
                        NK = 4 * Q + 4
                        probs = []
                        for j in range(NK):
                            sc = ps_sc.tile([P, 512], f32, tag="sc")
                            nc.tensor.matmul(
                                sc,
                                kT_sb[:, h, j * P : (j + 1) * P],
                                qT_sb[:, h, Q * 512 : (Q + 1) * 512],
                                start=True,
                                stop=True,
                            )
                            pj = probp.tile([P, 512], bf16, tag="probs")
                            nc.scalar.activation(out=pj, in_=sc, func=Exp)
                            r = j - 4 * Q
                            if r >= 0:
                                nc.vector.tensor_mul(pj, pj, mask_sb[:, r, :])
                            probs.append(pj)
                        pst = ps_tr.tile([P, 4, P], f32r, tag="tr")
                        for qt in range(4):
                            i = 4 * Q + qt
                            av = ps_av.tile([P, d + 1], f32, tag="av")
                            for j in range(i + 1):
                                nc.tensor.matmul(
                                    av,
                                    probs[j][:, qt * P : (qt + 1) * P],
                                    v_sb[:, j, h, :],
                                    start=(j == 0),
                                    stop=(j == i),
                                )
                            rec = smallp.tile([P, 1], f32, tag="rec")
                            nc.vector.reciprocal(rec, av[:, d : d + 1])
                            ao = smallp.tile([P, d], f32r, tag="ao")
                            nc.vector.tensor_scalar_mul(ao, av[:, 0:d], rec)
                            nc.tensor.transpose(pst[:, qt, :], ao, id_sb)
                        nc.vector.tensor_copy(
                            attoutT_sb[:, h, Q * 512 : (Q + 1) * 512],
                            pst.rearrange("p q e -> p (q e)"),
                        )

            # ---- stage 3: output projection ----
            with ExitStack() as s3:
                yp = s3.enter_context(tc.tile_pool(name="yp", bufs=4))
                for qt in range(ST):
                    for nchunk in range(4):
                        ps = ps_sc.tile([P, 512], f32, tag="sc")
                        for h in range(HPC):
                            nc.tensor.matmul(
                                ps,
                                attoutT_sb[:, h, qt * P : (qt + 1) * P],
                                wo_sb[:, h, nchunk * 512 : (nchunk + 1) * 512],
                                start=(h == 0),
                                stop=(h == HPC - 1),
                            )
                        yt = yp.tile([P, 512], f32, tag="y")
                        nc.scalar.copy(yt, ps)
                        nc.sync.dma_start(
                            out=y_r[:, qt, nchunk * 512 : (nchunk + 1) * 512], in_=yt
                        )
            s23.close()

    nc.compile()
    return nc


def _static_inputs():
    import ml_dtypes

    masks = np.zeros((4, P, 512), dtype=np.float32)
    kk = np.arange(P)[:, None]
    qq = np.arange(512)[None, :]
    for r in range(4):
        masks[r] = (P * r + kk <= qq).astype(np.float32)
    return masks.astype(ml_dtypes.bfloat16), np.eye(P, dtype=np.float32)


def make_in_maps(x, Wq, Wk, Wv, Wo):
    """Shard full inputs into 8 per-core input dicts."""
    bm, ident = _static_inputs()
    scale = 1.0 / math.sqrt(d)
    in_maps = []
    for c in range(N_CORES):
        b, g = divmod(c, 4)
        hs = g * HPC * d  # 512*g: rows of Wq for this head group
        in_maps.append(
            {
                "xT": np.ascontiguousarray(x[b].T),
                "wq": np.ascontiguousarray(Wq[hs : hs + 512, :].T) * np.float32(scale),
                "wk": np.ascontiguousarray(Wk[hs : hs + 512, :].T),
                "wv": np.ascontiguousarray(Wv[hs : hs + 512, :].T),
                "wo": np.ascontiguousarray(Wo[:, hs : hs + 512].T),
                "bm": bm,
                "ident": ident,
            }
        )
    return in_maps


def combine_results(results):
    """results: list of 8 dicts with 'y' [S, D] partials -> full [B, S, D]."""
    y = np.zeros((B, S, D), dtype=np.float32)
    for c in range(N_CORES):
        b = c // 4
        y[b] += results[c]["y"]
    return y


def _is_canonical_causal(attn_mask):
    m = np.asarray(attn_mask).reshape(S, S)
    iu = np.triu_indices(S, k=1)
    if not np.all(m[iu] <= -1e8):
        return False
    il = np.tril_indices(S, k=0)
    return np.all(m[il] == 0.0)


def _scores_safe(x, Wq, Wk):
    """Sampled bound on |scores| to make exp-without-max safe."""
    rng = np.random.default_rng(0)
    qi = rng.choice(S, 96, replace=False)
    ki = rng.choice(S, 384, replace=False)
    mx = 0.0
    for b in range(B):
        q = (x[b][qi] @ Wq.T) / math.sqrt(d)  # [96, D]
        k = x[b][ki] @ Wk.T  # [384, D]
        qh = q.reshape(96, H, d)
        kh = k.reshape(384, H, d)
        s = np.einsum("qhd,khd->hqk", qh, kh)
        mx = max(mx, float(np.abs(s).max()))
    return mx < 30.0


def _numpy_reference(x, attn_mask, Wq, Wk, Wv, Wo):
    out = np.zeros((B, S, D), dtype=np.float32)
    m = np.asarray(attn_mask, dtype=np.float32).reshape(S, S)
    for b in range(B):
        q = (x[b] @ Wq.T).reshape(S, H, d).transpose(1, 0, 2)
        k = (x[b] @ Wk.T).reshape(S, H, d).transpose(1, 0, 2)
        v = (x[b] @ Wv.T).reshape(S, H, d).transpose(1, 0, 2)
        q = q / np.float32(math.sqrt(d))
        att_out = np.zeros((H, S, d), dtype=np.float32)
        for h in range(H):
            s = q[h] @ k[h].T + m
            s = s - s.max(axis=-1, keepdims=True)
            p = np.exp(s)
            p /= p.sum(axis=-1, keepdims=True)
            att_out[h] = p @ v[h]
        out[b] = att_out.transpose(1, 0, 2).reshape(S, D) @ Wo.T
    return out


def kernel(x, attn_mask, Wq, Wk, Wv, Wo):
    x = np.asarray(x, dtype=np.float32)
    Wq = np.asarray(Wq, dtype=np.float32)
    Wk = np.asarray(Wk, dtype=np.float32)
    Wv = np.asarray(Wv, dtype=np.float32)
    Wo = np.asarray(Wo, dtype=np.float32)

    if not _is_canonical_causal(attn_mask) or not _scores_safe(x, Wq, Wk):
        return _numpy_reference(x, attn_mask, Wq, Wk, Wv, Wo)

    from concourse.bass_utils import run_bass_kernel_spmd

    if "nc" not in _CACHE:
        _CACHE["nc"] = _build_module()
    nc = _CACHE["nc"]

    in_maps = make_in_maps(x, Wq, Wk, Wv, Wo)
    res = run_bass_kernel_spmd(nc, in_maps, core_ids=list(range(N_CORES)))
    return combine_results(res.results)



# revision 36
# speedup vs baseline: 1.2386x; 1.2386x over previous
"""Causal self-attention (B=2, S=2048, D=2048, H=16) on 8 TRN2 NeuronCores.

Sharding: tensor-parallel over heads x data-parallel over batch.
Core c = b*4 + g handles batch b and heads 4g..4g+3 (head_dim=128).

Single-NEFF SPMD design (all-bf16 matmuls, wavefront schedule):
  - x is streamed from DRAM once (bf16, transposed layout xT [D, S]), in 4
    chunks of 512 tokens.  Per chunk: q/k projections per head (qT/kT [d, S]
    bf16) and v projection (natural [tok, d] bf16), all bf16 matmuls with
    fp32 PSUM accumulation.
  - After chunk c, attention "row" c runs for all 4 heads: transposed scores
    sT[j] = k_j @ qT_block (f32 PSUM, trimmed to the causal triangle at
    128-col granularity), exp on ACT -> probs bf16, binary causal mask on
    diagonal-strip tiles (DVE), then avT[d, 512q] += v_j.T @ probs_j on PE.
    Softmax denominators come from near-free ap=1 matmuls
    (probs_j[:,qt].T @ ones).  Normalization: reciprocal (DVE), PE-transpose
    of the [128,4] rec vector, partition-broadcast (GPSIMD) to a [128,512]
    scale tile, one DVE multiply into attoutT bf16.
  - Stage-3 output projection chunks (y[qt] = sum_h attoutT_h.T @ Wo_h) are
    interleaved as PE "filler" work one row behind attention, so the ACT exp
    stream never stalls the PE.  y is written bf16; the host accumulates the
    4 per-core partials per batch in fp32.

Softmax skips the max-subtraction (scores are O(5..30) for the expected
input distribution; a host-side sampling guard falls back to a numpy
reference if scores could overflow exp, or if the mask is not the canonical
causal mask).
"""

import math
from contextlib import ExitStack

import numpy as np

B = 2
S = 2048
D = 2048
H = 16
HPC = 4  # heads per core
d = 128  # head dim
N_CORES = 8
P = 128
DK = D // P  # 16 contraction tiles
ST = S // P  # 16 token tiles
CH = S // 512  # 4 chunks of 512 tokens
PROBS_BUFS = 20

_CACHE = {}


def _build_module():
    import concourse.mybir as mybir
    import concourse.tile as tile
    from concourse import bacc

    f32 = mybir.dt.float32
    f32r = mybir.dt.float32r
    bf16 = mybir.dt.bfloat16
    Exp = mybir.ActivationFunctionType.Exp

    nc = bacc.Bacc("TRN2", target_bir_lowering=False, debug=False)

    xT = nc.dram_tensor("xT", [D, S], bf16, kind="ExternalInput")
    wq = nc.dram_tensor("wq", [D, HPC * d], bf16, kind="ExternalInput")
    wk = nc.dram_tensor("wk", [D, HPC * d], bf16, kind="ExternalInput")
    wv = nc.dram_tensor("wv", [D, HPC * d], bf16, kind="ExternalInput")
    wo = nc.dram_tensor("wo", [HPC * d, D], bf16, kind="ExternalInput")
    bm = nc.dram_tensor("bm", [4, P, 512], bf16, kind="ExternalInput")
    identb = nc.dram_tensor("identb", [P, P], f32, kind="ExternalInput")
    y = nc.dram_tensor("y", [S, D], bf16, kind="ExternalOutput")

    xT_r = xT.ap().rearrange("(t p) s -> p t s", p=P)
    wq_r = wq.ap().rearrange("(t p) m -> p t m", p=P)
    wk_r = wk.ap().rearrange("(t p) m -> p t m", p=P)
    wv_r = wv.ap().rearrange("(t p) m -> p t m", p=P)
    wo_r = wo.ap().rearrange("(t p) n -> p t n", p=P)
    y_r = y.ap().rearrange("(t p) n -> p t n", p=P)

    with tile.TileContext(nc) as tc, ExitStack() as top:
        consts = top.enter_context(tc.tile_pool(name="consts", bufs=1))
        wpool = top.enter_context(tc.tile_pool(name="wpool", bufs=1))
        wop = top.enter_context(tc.tile_pool(name="wop", bufs=1))
        xp = top.enter_context(tc.tile_pool(name="xp", bufs=2))
        qkp = top.enter_context(tc.tile_pool(name="qkp", bufs=1))
        vp = top.enter_context(tc.tile_pool(name="vp", bufs=1))
        aop = top.enter_context(tc.tile_pool(name="aop", bufs=1))
        probp = top.enter_context(tc.tile_pool(name="probp", bufs=PROBS_BUFS))
        smallp = top.enter_context(tc.tile_pool(name="smallp", bufs=2))
        ysp = top.enter_context(tc.tile_pool(name="ysp", bufs=8))
        # PSUM: pp(2) + sc(3) + av(2) + scr(1) = 8 banks
        pp = top.enter_context(tc.tile_pool(name="pp", bufs=2, space="PSUM"))
        scp = top.enter_context(tc.tile_pool(name="scp", bufs=3, space="PSUM"))
        avp = top.enter_context(tc.tile_pool(name="avp", bufs=2, space="PSUM"))
        scrp = top.enter_context(tc.tile_pool(name="scrp", bufs=1, space="PSUM"))

        bm_sb = consts.tile([P, 4, 512], bf16, tag="bm")
        id_sb = consts.tile([P, P], f32, tag="identb")
        ones_col = consts.tile([P, 1], bf16, tag="ones")
        wq_sb = wpool.tile([P, DK, HPC * d], bf16, tag="wq")
        wk_sb = wpool.tile([P, DK, HPC * d], bf16, tag="wk")
        wv_sb = wpool.tile([P, DK, HPC * d], bf16, tag="wv")
        wo_sb = wop.tile([P, HPC, D], bf16, tag="wo")
        qT_sb = qkp.tile([P, HPC, S], bf16, tag="qT")
        kT_sb = qkp.tile([P, HPC, S], bf16, tag="kT")
        v_sb = vp.tile([P, ST, HPC, d], bf16, tag="v")
        attoutT_sb = aop.tile([P, HPC, S], bf16, tag="attoutT")
        scr = scrp.tile([P, 512], f32, tag="scr")  # softmax denominators, cols 0:4

        nc.gpsimd.memset(ones_col, 1.0)

        # ---- initial DMAs (SP ring, priority order) ----
        xc_tiles = [None] * CH

        def dma_x_chunk(c):
            t = xp.tile([P, DK, 512], bf16, tag="xc", name=f"xc{c}")
            for p4 in range(4):
                nc.sync.dma_start(
                    out=t[:, 4 * p4 : 4 * p4 + 4, :],
                    in_=xT_r[:, 4 * p4 : 4 * p4 + 4, c * 512 : (c + 1) * 512],
                )
            xc_tiles[c] = t

        xc0 = xp.tile([P, DK, 512], bf16, tag="xc", name="xc0")
        xc_tiles[0] = xc0
        # interleave wq/x0 in small pieces: the tile-major chunk-0 schedule
        # below consumes the stream at line rate
        bounds = [0, 1, 2, 4, 6, 8, 10, 12, 14, 16]
        for a, b in zip(bounds[:-1], bounds[1:]):
            nc.sync.dma_start(out=wq_sb[:, a:b, :], in_=wq_r[:, a:b, :])
            nc.sync.dma_start(out=xc0[:, a:b, :], in_=xT_r[:, a:b, 0:512])
        for w_sb, w_r in ((wk_sb, wk_r), (wv_sb, wv_r)):
            for p2 in range(8):
                nc.sync.dma_start(
                    out=w_sb[:, 2 * p2 : 2 * p2 + 2, :],
                    in_=w_r[:, 2 * p2 : 2 * p2 + 2, :],
                )
        dma_x_chunk(1)
        nc.sync.dma_start(out=bm_sb, in_=bm.ap().rearrange("r p m -> p r m"))
        nc.sync.dma_start(out=id_sb, in_=identb.ap())
        for p4 in range(4):
            nc.sync.dma_start(out=wo_sb[:, p4, :], in_=wo_r[:, p4, :])

        # ---- work units ----
        def proj_group(c, kind, idx):
            """kind: 'q'/'k' with idx=head, 'v' with idx=token-subtile."""
            xc = xc_tiles[c]
            ps = pp.tile([P, 512], f32, tag="pp", name="psp")
            if kind == "v":
                for kk in range(DK):
                    nc.tensor.matmul(
                        ps,
                        xc[:, kk, idx * P : (idx + 1) * P],
                        wv_sb[:, kk, :],
                        start=(kk == 0),
                        stop=(kk == DK - 1),
                    )
                nc.vector.tensor_copy(
                    v_sb[:, c * 4 + idx, :, :],
                    ps.rearrange("p (h e) -> p h e", h=HPC),
                )
            else:
                w_sb = wq_sb if kind == "q" else wk_sb
                dest = qT_sb if kind == "q" else kT_sb
                for kk in range(DK):
                    nc.tensor.matmul(
                        ps,
                        w_sb[:, kk, idx * d : (idx + 1) * d],
                        xc[:, kk, :],
                        start=(kk == 0),
                        stop=(kk == DK - 1),
                    )
                nc.vector.tensor_copy(dest[:, idx, c * 512 : (c + 1) * 512], ps)

        st3_state = {"mode": "dve", "alt": 0}

        def st3_chunk(qt, nch):
            ps = pp.tile([P, 512], f32, tag="pp", name="psy")
            for h in range(HPC):
                nc.tensor.matmul(
                    ps,
                    attoutT_sb[:, h, qt * P : (qt + 1) * P],
                    wo_sb[:, h, nch * 512 : (nch + 1) * 512],
                    start=(h == 0),
                    stop=(h == HPC - 1),
                )
            yt = ysp.tile([P, 512], bf16, tag="yt", name="yt")
            st3_state["alt"] += 1
            if st3_state["mode"] == "mix" and st3_state["alt"] % 2 == 0:
                nc.scalar.copy(yt, ps)
            else:
                nc.vector.tensor_copy(yt, ps)
            nc.sync.dma_start(out=y_r[:, qt, nch * 512 : (nch + 1) * 512], in_=yt)

        # filler fifo of thunks
        fifo = []

        def filler(n=1):
            for _ in range(n):
                if fifo:
                    fifo.pop(0)()

        def emit_aux(blk):
            """Normalization tail of a finished block: PE-transpose the four
            rec columns onto partition 0 of scr (forming a flat [1,512]),
            copy to SBUF, partition-broadcast to all 128 partitions, then one
            DVE multiply applies 1/den along the free (q) axis of avT."""
            c, h, avT, rec = blk
            recT_ps = scp.tile([P, 512], f32, tag="sc", name="recT_ps")
            for qt in range(4):
                nc.tensor.matmul(
                    recT_ps[0:1, qt * P : (qt + 1) * P],
                    rec[:, qt : qt + 1],
                    id_sb,
                    is_transpose=True,
                    start=(qt == 0),
                    stop=(qt == 3),
                )
            recT = smallp.tile([1, 512], bf16, tag="recT", name="recT")
            nc.vector.tensor_copy(recT, recT_ps[0:1, 0:512])
            filler()
            rb = smallp.tile([P, 512], bf16, tag="rb", name="rb")
            nc.gpsimd.partition_broadcast(rb, recT)
            nc.vector.tensor_mul(
                attoutT_sb[:, h, c * 512 : (c + 1) * 512], avT, rb
            )

        pending = [None]  # block awaiting its normalization tail

        def attn_block(c, h):
            NK = 4 * c + 4
            step = 4
            probs = []
            for j in range(NK):
                r = j - 4 * c
                lo = P * r if r > 0 else 0
                sc = scp.tile([P, 512], f32, tag="sc", name="sc")
                nc.tensor.matmul(
                    sc[:, lo:512],
                    kT_sb[:, h, j * P : (j + 1) * P],
                    qT_sb[:, h, c * 512 + lo : (c + 1) * 512],
                    start=True,
                    stop=True,
                )
                pj = probp.tile([P, 512], bf16, tag="probs", name="pj")
                nc.scalar.activation(out=pj[:, lo:512], in_=sc[:, lo:512], func=Exp)
                if r >= 0:
                    # trimmed: no consumer ever reads pj[:, :lo]
                    nc.vector.tensor_mul(
                        pj[:, lo:512], pj[:, lo:512], bm_sb[:, r, lo:512]
                    )
                probs.append(pj)
                if j % step == step - 1:
                    filler()
                if j == 7 and pending[0] is not None:
                    # flush early: queues the aux's DVE work ahead of this
                    # block's later masks, shortening the chain's latency
                    emit_aux(pending[0])
                    pending[0] = None
            if pending[0] is not None:
                emit_aux(pending[0])
                pending[0] = None
            avT = avp.tile([P, 512], f32, tag="av", name="avT")
            for j in range(NK):
                r = j - 4 * c
                lo = P * r if r > 0 else 0
                nc.tensor.matmul(
                    avT[:, lo:512],
                    v_sb[:, j, h, :],
                    probs[j][:, lo:512],
                    start=(j == 0),
                    stop=(j == NK - 1),
                )
                for qt in range(max(r, 0), 4):
                    # all four chains share one PSUM zero region (the whole
                    # bank): only the first matmul starts it, only the last
                    # one stops it
                    nc.tensor.matmul(
                        scr[:, qt : qt + 1],
                        probs[j][:, qt * P : (qt + 1) * P],
                        ones_col,
                        start=(j == 0 and qt == max(r, 0)),
                        stop=(j == NK - 1 and qt == 3),
                    )
                if j % step == step - 1:
                    filler()
            rec = smallp.tile([P, 4], f32, tag="rec", name="rec")
            nc.vector.reciprocal(rec, scr[:, 0:4])
            pending[0] = (c, h, avT, rec)

        # ---- main wavefront ----
        # chunk 0 runs tile-major: four open PSUM chains consume each weight/x
        # tile as it lands, so the PE tracks the startup DMA stream
        for kind in ("q", "k", "v"):
            chains = [
                pp.tile([P, 512], f32, tag="pp", name="c0ps"),
                pp.tile([P, 512], f32, tag="pp", name="c0ps"),
                scp.tile([P, 512], f32, tag="sc", name="c0ps"),
                scp.tile([P, 512], f32, tag="sc", name="c0ps"),
            ]
            w_sb = {"q": wq_sb, "k": wk_sb, "v": wv_sb}[kind]
            for t in range(DK):
                for i in range(4):
                    if kind == "v":
                        nc.tensor.matmul(
                            chains[i],
                            xc0[:, t, i * P : (i + 1) * P],
                            wv_sb[:, t, :],
                            start=(t == 0),
                            stop=(t == DK - 1),
                        )
                    else:
                        nc.tensor.matmul(
                            chains[i],
                            w_sb[:, t, i * d : (i + 1) * d],
                            xc0[:, t, :],
                            start=(t == 0),
                            stop=(t == DK - 1),
                        )
            for i in range(4):
                if kind == "v":
                    nc.vector.tensor_copy(
                        v_sb[:, i, :, :],
                        chains[i].rearrange("p (h e) -> p h e", h=HPC),
                    )
                else:
                    dest = qT_sb if kind == "q" else kT_sb
                    nc.vector.tensor_copy(dest[:, i, 0:512], chains[i])

        for c in range(CH):
            # queue fillers: next chunk's projections first
            if c + 1 < CH:
                if c + 2 < CH:
                    fifo.append(lambda cc=c + 2: dma_x_chunk(cc))
                for kind, idx in _chunk_order():
                    fifo.append(lambda k=kind, i=idx, cc=c + 1: proj_group(cc, k, i))
            for h in range(HPC):
                attn_block(c, h)
                if h == 0 and c >= 1:
                    # row c-1's attoutT is fully normalized once block (c, h0)
                    # has flushed the pending aux -> stage-3 row c-1 is safe
                    for qt in range(4 * (c - 1), 4 * c):
                        for nch in range(4):
                            fifo.append(lambda q=qt, n=nch: st3_chunk(q, n))
            if c == CH - 2:
                # hold a few stage-3 fillers back for row 3's first block
                while len(fifo) > 4:
                    fifo.pop(0)()
            else:
                while fifo:
                    fifo.pop(0)()

        # Final stage-3 row with a deep pipeline: borrow the (now idle)
        # scores banks, start the first 5 chunks with h0-h2 partial chains
        # so the h3 matmuls land after the last normalization, and alternate
        # copies DVE/ACT.
        STG = 6
        staged_pools = [
            (pp, "pp"), (scp, "sc"), (pp, "pp"), (scp, "sc"), (scp, "sc"), (avp, "av"),
        ]

        def st3_tail(k, ps=None):
            qt, nch = 4 * (CH - 1) + k // 4, k % 4
            if ps is None:
                pool, tag = (scp, "sc") if k % 2 else (pp, "pp")
                ps = pool.tile([P, 512], f32, tag=tag, name="psy")
                h0 = 0
            else:
                h0 = HPC - 1
            for h in range(h0, HPC):
                nc.tensor.matmul(
                    ps,
                    attoutT_sb[:, h, qt * P : (qt + 1) * P],
                    wo_sb[:, h, nch * 512 : (nch + 1) * 512],
                    start=(h == 0),
                    stop=(h == HPC - 1),
                )
            yt = ysp.tile([P, 512], bf16, tag="yt", name="yt")
            if k % 2:
                nc.vector.tensor_copy(yt, ps)
            else:
                nc.scalar.copy(yt, ps)
            nc.sync.dma_start(out=y_r[:, qt, nch * 512 : (nch + 1) * 512], in_=yt)

        staged = []
        for k in range(STG):
            qt, nch = 4 * (CH - 1) + k // 4, k % 4
            pool, tag = staged_pools[k]
            ps = pool.tile([P, 512], f32, tag=tag, name="psy")
            for h in range(HPC - 1):
                nc.tensor.matmul(
                    ps,
                    attoutT_sb[:, h, qt * P : (qt + 1) * P],
                    wo_sb[:, h, nch * 512 : (nch + 1) * 512],
                    start=(h == 0),
                    stop=False,
                )
            staged.append(ps)
            if k == 0 and pending[0] is not None:
                emit_aux(pending[0])
                pending[0] = None
        for k in range(STG):
            st3_tail(k, ps=staged[k])
        for k in range(STG, 16):
            st3_tail(k)

    nc.compile()
    return nc


def _chunk_order():
    # q-heads first (reuse wq while wk/wv stream in at startup), then k0 so
    # the row's first scores have kT, then v, then the remaining k-heads
    order = [("q", h) for h in range(HPC)]
    order += [("k", 0)]
    order += [("v", t) for t in range(4)]
    order += [("k", h) for h in range(1, HPC)]
    return order


def _static_inputs():
    import ml_dtypes

    masks = np.zeros((4, P, 512), dtype=np.float32)
    kk = np.arange(P)[:, None]
    qq = np.arange(512)[None, :]
    for r in range(4):
        masks[r] = (P * r + kk <= qq).astype(np.float32)
    return masks.astype(ml_dtypes.bfloat16), np.eye(P, dtype=np.float32)


def make_in_maps(x, Wq, Wk, Wv, Wo):
    """Shard full inputs into 8 per-core input dicts (bf16)."""
    import ml_dtypes

    bf = ml_dtypes.bfloat16
    bm, identb = _static_inputs()
    scale = 1.0 / math.sqrt(d)
    in_maps = []
    for c in range(N_CORES):
        b, g = divmod(c, 4)
        hs = g * HPC * d  # 512*g: rows of Wq for this head group
        in_maps.append(
            {
                "xT": np.ascontiguousarray(x[b].T).astype(bf),
                "wq": (np.ascontiguousarray(Wq[hs : hs + 512, :].T) * np.float32(scale)).astype(bf),
                "wk": np.ascontiguousarray(Wk[hs : hs + 512, :].T).astype(bf),
                "wv": np.ascontiguousarray(Wv[hs : hs + 512, :].T).astype(bf),
                "wo": np.ascontiguousarray(Wo[:, hs : hs + 512].T).astype(bf),
                "bm": bm,
                "identb": identb,
            }
        )
    return in_maps


def combine_results(results):
    """results: list of 8 dicts with 'y' [S, D] bf16 partials -> full [B, S, D]."""
    y = np.zeros((B, S, D), dtype=np.float32)
    for c in range(N_CORES):
        b = c // 4
        y[b] += np.asarray(results[c]["y"], dtype=np.float32)
    return y


def _is_canonical_causal(attn_mask):
    m = np.asarray(attn_mask).reshape(S, S)
    iu = np.triu_indices(S, k=1)
    if not np.all(m[iu] <= -1e8):
        return False
    il = np.tril_indices(S, k=0)
    return np.all(m[il] == 0.0)


def _scores_safe(x, Wq, Wk):
    """Sampled bound on |scores| to make exp-without-max safe."""
    rng = np.random.default_rng(0)
    qi = rng.choice(S, 96, replace=False)
    ki = rng.choice(S, 384, replace=False)
    mx = 0.0
    for b in range(B):
        q = (x[b][qi] @ Wq.T) / math.sqrt(d)  # [96, D]
        k = x[b][ki] @ Wk.T  # [384, D]
        qh = q.reshape(96, H, d)
        kh = k.reshape(384, H, d)
        s = np.einsum("qhd,khd->hqk", qh, kh)
        mx = max(mx, float(np.abs(s).max()))
    return mx < 30.0


def _numpy_reference(x, attn_mask, Wq, Wk, Wv, Wo):
    out = np.zeros((B, S, D), dtype=np.float32)
    m = np.asarray(attn_mask, dtype=np.float32).reshape(S, S)
    for b in range(B):
        q = (x[b] @ Wq.T).reshape(S, H, d).transpose(1, 0, 2)
        k = (x[b] @ Wk.T).reshape(S, H, d).transpose(1, 0, 2)
        v = (x[b] @ Wv.T).reshape(S, H, d).transpose(1, 0, 2)
        q = q / np.float32(math.sqrt(d))
        att_out = np.zeros((H, S, d), dtype=np.float32)
        for h in range(H):
            s = q[h] @ k[h].T + m
            s = s - s.max(axis=-1, keepdims=True)
            p = np.exp(s)
            p /= p.sum(axis=-1, keepdims=True)
            att_out[h] = p @ v[h]
        out[b] = att_out.transpose(1, 0, 2).reshape(S, D) @ Wo.T
    return out


def kernel(x, attn_mask, Wq, Wk, Wv, Wo):
    x = np.asarray(x, dtype=np.float32)
    Wq = np.asarray(Wq, dtype=np.float32)
    Wk = np.asarray(Wk, dtype=np.float32)
    Wv = np.asarray(Wv, dtype=np.float32)
    Wo = np.asarray(Wo, dtype=np.float32)

    if not _is_canonical_causal(attn_mask) or not _scores_safe(x, Wq, Wk):
        return _numpy_reference(x, attn_mask, Wq, Wk, Wv, Wo)

    from concourse.bass_utils import run_bass_kernel_spmd

    if "nc" not in _CACHE:
        _CACHE["nc"] = _build_module()
    nc = _CACHE["nc"]

    in_maps = make_in_maps(x, Wq, Wk, Wv, Wo)
    res = run_bass_kernel_spmd(nc, in_maps, core_ids=list(range(N_CORES)))
    return combine_results(res.results)


# revision 43
# speedup vs baseline: 1.2434x; 1.0039x over previous
"""Causal self-attention (B=2, S=2048, D=2048, H=16) on 8 TRN2 NeuronCores.

Sharding: tensor-parallel over heads x data-parallel over batch.
Core c = b*4 + g handles batch b and heads 4g..4g+3 (head_dim=128).

Single-NEFF SPMD design (all-bf16 matmuls, wavefront schedule):
  - x is streamed from DRAM once (bf16, transposed layout xT [D, S]), in 4
    chunks of 512 tokens.  Per chunk: q/k projections per head (qT/kT [d, S]
    bf16) and v projection (natural [tok, d] bf16), all bf16 matmuls with
    fp32 PSUM accumulation.
  - After chunk c, attention "row" c runs for all 4 heads: transposed scores
    sT[j] = k_j @ qT_block (f32 PSUM, trimmed to the causal triangle at
    128-col granularity), exp on ACT -> probs bf16, binary causal mask on
    diagonal-strip tiles (DVE), then avT[d, 512q] += v_j.T @ probs_j on PE.
    Softmax denominators come from near-free ap=1 matmuls
    (probs_j[:,qt].T @ ones).  Normalization: reciprocal (DVE), PE-transpose
    of the [128,4] rec vector, partition-broadcast (GPSIMD) to a [128,512]
    scale tile, one DVE multiply into attoutT bf16.
  - Stage-3 output projection chunks (y[qt] = sum_h attoutT_h.T @ Wo_h) are
    interleaved as PE "filler" work one row behind attention, so the ACT exp
    stream never stalls the PE.  y is written bf16; the host accumulates the
    4 per-core partials per batch in fp32.

Softmax skips the max-subtraction (scores are O(5..30) for the expected
input distribution; a host-side sampling guard falls back to a numpy
reference if scores could overflow exp, or if the mask is not the canonical
causal mask).
"""

import math
from contextlib import ExitStack

import numpy as np

B = 2
S = 2048
D = 2048
H = 16
HPC = 4  # heads per core
d = 128  # head dim
N_CORES = 8
P = 128
DK = D // P  # 16 contraction tiles
ST = S // P  # 16 token tiles
CH = S // 512  # 4 chunks of 512 tokens
PROBS_BUFS = 20

_CACHE = {}


def _build_module():
    import concourse.mybir as mybir
    import concourse.tile as tile
    from concourse import bacc

    f32 = mybir.dt.float32
    f32r = mybir.dt.float32r
    bf16 = mybir.dt.bfloat16
    Exp = mybir.ActivationFunctionType.Exp

    nc = bacc.Bacc("TRN2", target_bir_lowering=False, debug=False)

    xT = nc.dram_tensor("xT", [D, S], bf16, kind="ExternalInput")
    wq = nc.dram_tensor("wq", [D, HPC * d], bf16, kind="ExternalInput")
    wk = nc.dram_tensor("wk", [D, HPC * d], bf16, kind="ExternalInput")
    wv = nc.dram_tensor("wv", [D, HPC * d], bf16, kind="ExternalInput")
    wo = nc.dram_tensor("wo", [HPC * d, D], bf16, kind="ExternalInput")
    bm = nc.dram_tensor("bm", [4, P, 512], bf16, kind="ExternalInput")
    identb = nc.dram_tensor("identb", [P, P], f32, kind="ExternalInput")
    y = nc.dram_tensor("y", [S, D], bf16, kind="ExternalOutput")

    xT_r = xT.ap().rearrange("(t p) s -> p t s", p=P)
    wq_r = wq.ap().rearrange("(t p) m -> p t m", p=P)
    wk_r = wk.ap().rearrange("(t p) m -> p t m", p=P)
    wv_r = wv.ap().rearrange("(t p) m -> p t m", p=P)
    wo_r = wo.ap().rearrange("(t p) n -> p t n", p=P)
    y_r = y.ap().rearrange("(t p) n -> p t n", p=P)

    with tile.TileContext(nc) as tc, ExitStack() as top:
        consts = top.enter_context(tc.tile_pool(name="consts", bufs=1))
        wpool = top.enter_context(tc.tile_pool(name="wpool", bufs=1))
        wop = top.enter_context(tc.tile_pool(name="wop", bufs=1))
        xp = top.enter_context(tc.tile_pool(name="xp", bufs=2))
        qkp = top.enter_context(tc.tile_pool(name="qkp", bufs=1))
        vp = top.enter_context(tc.tile_pool(name="vp", bufs=1))
        aop = top.enter_context(tc.tile_pool(name="aop", bufs=1))
        probp = top.enter_context(tc.tile_pool(name="probp", bufs=PROBS_BUFS))
        smallp = top.enter_context(tc.tile_pool(name="smallp", bufs=3))
        ysp = top.enter_context(tc.tile_pool(name="ysp", bufs=8))
        # PSUM: pp(2) + sc(3) + av(2) + scr(1) = 8 banks
        pp = top.enter_context(tc.tile_pool(name="pp", bufs=2, space="PSUM"))
        scp = top.enter_context(tc.tile_pool(name="scp", bufs=3, space="PSUM"))
        avp = top.enter_context(tc.tile_pool(name="avp", bufs=2, space="PSUM"))
        scrp = top.enter_context(tc.tile_pool(name="scrp", bufs=1, space="PSUM"))

        bm_sb = consts.tile([P, 4, 512], bf16, tag="bm")
        id_sb = consts.tile([P, P], f32, tag="identb")
        ones_col = consts.tile([P, 1], bf16, tag="ones")
        wq_sb = wpool.tile([P, DK, HPC * d], bf16, tag="wq")
        wk_sb = wpool.tile([P, DK, HPC * d], bf16, tag="wk")
        wv_sb = wpool.tile([P, DK, HPC * d], bf16, tag="wv")
        wo_sb = wop.tile([P, HPC, D], bf16, tag="wo")
        qT_sb = qkp.tile([P, HPC, S], bf16, tag="qT")
        kT_sb = qkp.tile([P, HPC, S], bf16, tag="kT")
        v_sb = vp.tile([P, ST, HPC, d], bf16, tag="v")
        attoutT_sb = aop.tile([P, HPC, S], bf16, tag="attoutT")
        scr = scrp.tile([P, 512], f32, tag="scr")  # softmax denominators, cols 0:4

        nc.gpsimd.memset(ones_col, 1.0)

        # ---- initial DMAs (SP ring, priority order) ----
        xc_tiles = [None] * CH

        def dma_x_chunk(c):
            t = xp.tile([P, DK, 512], bf16, tag="xc", name=f"xc{c}")
            for p4 in range(4):
                nc.sync.dma_start(
                    out=t[:, 4 * p4 : 4 * p4 + 4, :],
                    in_=xT_r[:, 4 * p4 : 4 * p4 + 4, c * 512 : (c + 1) * 512],
                )
            xc_tiles[c] = t

        xc0 = xp.tile([P, DK, 512], bf16, tag="xc", name="xc0")
        xc_tiles[0] = xc0
        # interleave wq/x0 in small pieces: the tile-major chunk-0 schedule
        # below consumes the stream at line rate
        bounds = [0, 1, 2, 4, 6, 8, 10, 12, 14, 16]
        for a, b in zip(bounds[:-1], bounds[1:]):
            nc.sync.dma_start(out=wq_sb[:, a:b, :], in_=wq_r[:, a:b, :])
            nc.sync.dma_start(out=xc0[:, a:b, :], in_=xT_r[:, a:b, 0:512])
        for w_sb, w_r in ((wk_sb, wk_r), (wv_sb, wv_r)):
            for p2 in range(8):
                nc.sync.dma_start(
                    out=w_sb[:, 2 * p2 : 2 * p2 + 2, :],
                    in_=w_r[:, 2 * p2 : 2 * p2 + 2, :],
                )
        dma_x_chunk(1)
        nc.sync.dma_start(out=bm_sb, in_=bm.ap().rearrange("r p m -> p r m"))
        nc.sync.dma_start(out=id_sb, in_=identb.ap())
        for p4 in range(4):
            nc.sync.dma_start(out=wo_sb[:, p4, :], in_=wo_r[:, p4, :])

        # ---- work units ----
        def proj_group(c, kind, idx):
            """kind: 'q'/'k' with idx=head, 'v' with idx=token-subtile."""
            xc = xc_tiles[c]
            ps = pp.tile([P, 512], f32, tag="pp", name="psp")
            if kind == "v":
                for kk in range(DK):
                    nc.tensor.matmul(
                        ps,
                        xc[:, kk, idx * P : (idx + 1) * P],
                        wv_sb[:, kk, :],
                        start=(kk == 0),
                        stop=(kk == DK - 1),
                    )
                nc.vector.tensor_copy(
                    v_sb[:, c * 4 + idx, :, :],
                    ps.rearrange("p (h e) -> p h e", h=HPC),
                )
            else:
                w_sb = wq_sb if kind == "q" else wk_sb
                dest = qT_sb if kind == "q" else kT_sb
                for kk in range(DK):
                    nc.tensor.matmul(
                        ps,
                        w_sb[:, kk, idx * d : (idx + 1) * d],
                        xc[:, kk, :],
                        start=(kk == 0),
                        stop=(kk == DK - 1),
                    )
                nc.vector.tensor_copy(dest[:, idx, c * 512 : (c + 1) * 512], ps)

        st3_state = {"mode": "dve", "alt": 0}

        def st3_chunk(qt, nch):
            ps = pp.tile([P, 512], f32, tag="pp", name="psy")
            for h in range(HPC):
                nc.tensor.matmul(
                    ps,
                    attoutT_sb[:, h, qt * P : (qt + 1) * P],
                    wo_sb[:, h, nch * 512 : (nch + 1) * 512],
                    start=(h == 0),
                    stop=(h == HPC - 1),
                )
            yt = ysp.tile([P, 512], bf16, tag="yt", name="yt")
            st3_state["alt"] += 1
            if st3_state["mode"] == "mix" and st3_state["alt"] % 2 == 0:
                nc.scalar.copy(yt, ps)
            else:
                nc.vector.tensor_copy(yt, ps)
            nc.sync.dma_start(out=y_r[:, qt, nch * 512 : (nch + 1) * 512], in_=yt)

        # filler fifo of thunks
        fifo = []

        def filler(n=1):
            for _ in range(n):
                if fifo:
                    fifo.pop(0)()

        def emit_aux(blk):
            """Normalization tail of a finished block: one PE transpose of the
            [128,4] rec vector, DVE copy to SBUF, a tiny SBUF->SBUF DMA that
            flattens [4,128] onto partition 0, partition-broadcast (GPSIMD) to
            [128,512], then one DVE multiply applies 1/den along the free (q)
            axis of avT."""
            c, h, avT, rec = blk
            recT_ps = scp.tile([P, 512], f32, tag="sc", name="recT_ps")
            nc.tensor.transpose(recT_ps[0:4, 0:P], rec, id_sb)
            recT4 = smallp.tile([4, P], bf16, tag="recT4", name="recT4")
            nc.vector.tensor_copy(recT4, recT_ps[0:4, 0:P])
            recT = smallp.tile([1, 512], bf16, tag="recT", name="recT")
            nc.scalar.dma_start(out=recT, in_=recT4)
            filler()
            rb = smallp.tile([P, 512], bf16, tag="rb", name="rb")
            nc.gpsimd.partition_broadcast(rb, recT)
            nc.vector.tensor_mul(
                attoutT_sb[:, h, c * 512 : (c + 1) * 512], avT, rb
            )

        pending = [None]  # block awaiting its normalization tail

        def attn_block(c, h):
            NK = 4 * c + 4
            step = 4
            probs = []
            for j in range(NK):
                r = j - 4 * c
                lo = P * r if r > 0 else 0
                sc = scp.tile([P, 512], f32, tag="sc", name="sc")
                nc.tensor.matmul(
                    sc[:, lo:512],
                    kT_sb[:, h, j * P : (j + 1) * P],
                    qT_sb[:, h, c * 512 + lo : (c + 1) * 512],
                    start=True,
                    stop=True,
                )
                pj = probp.tile([P, 512], bf16, tag="probs", name="pj")
                nc.scalar.activation(out=pj[:, lo:512], in_=sc[:, lo:512], func=Exp)
                if r >= 0:
                    # trimmed: no consumer ever reads pj[:, :lo]
                    nc.vector.tensor_mul(
                        pj[:, lo:512], pj[:, lo:512], bm_sb[:, r, lo:512]
                    )
                probs.append(pj)
                if j % step == step - 1:
                    filler()
                if j == 7 and pending[0] is not None:
                    # flush early: queues the aux's DVE work ahead of this
                    # block's later masks, shortening the chain's latency
                    emit_aux(pending[0])
                    pending[0] = None
            if pending[0] is not None:
                emit_aux(pending[0])
                pending[0] = None
            avT = avp.tile([P, 512], f32, tag="av", name="avT")
            for j in range(NK):
                r = j - 4 * c
                lo = P * r if r > 0 else 0
                nc.tensor.matmul(
                    avT[:, lo:512],
                    v_sb[:, j, h, :],
                    probs[j][:, lo:512],
                    start=(j == 0),
                    stop=(j == NK - 1),
                )
                for qt in range(max(r, 0), 4):
                    # all four chains share one PSUM zero region (the whole
                    # bank): only the first matmul starts it, only the last
                    # one stops it
                    nc.tensor.matmul(
                        scr[:, qt : qt + 1],
                        probs[j][:, qt * P : (qt + 1) * P],
                        ones_col,
                        start=(j == 0 and qt == max(r, 0)),
                        stop=(j == NK - 1 and qt == 3),
                    )
                if j % step == step - 1:
                    filler()
            rec = smallp.tile([P, 4], f32, tag="rec", name="rec")
            nc.vector.reciprocal(rec, scr[:, 0:4])
            pending[0] = (c, h, avT, rec)

        # ---- main wavefront ----
        # chunk 0 runs tile-major: four open PSUM chains consume each weight/x
        # tile as it lands, so the PE tracks the startup DMA stream
        for kind in ("q", "k", "v"):
            chains = [
                pp.tile([P, 512], f32, tag="pp", name="c0ps"),
                pp.tile([P, 512], f32, tag="pp", name="c0ps"),
                scp.tile([P, 512], f32, tag="sc", name="c0ps"),
                scp.tile([P, 512], f32, tag="sc", name="c0ps"),
            ]
            w_sb = {"q": wq_sb, "k": wk_sb, "v": wv_sb}[kind]
            for t in range(DK):
                for i in range(4):
                    if kind == "v":
                        nc.tensor.matmul(
                            chains[i],
                            xc0[:, t, i * P : (i + 1) * P],
                            wv_sb[:, t, :],
                            start=(t == 0),
                            stop=(t == DK - 1),
                        )
                    else:
                        nc.tensor.matmul(
                            chains[i],
                            w_sb[:, t, i * d : (i + 1) * d],
                            xc0[:, t, :],
                            start=(t == 0),
                            stop=(t == DK - 1),
                        )
            for i in range(4):
                if kind == "v":
                    nc.vector.tensor_copy(
                        v_sb[:, i, :, :],
                        chains[i].rearrange("p (h e) -> p h e", h=HPC),
                    )
                else:
                    dest = qT_sb if kind == "q" else kT_sb
                    nc.vector.tensor_copy(dest[:, i, 0:512], chains[i])

        for c in range(CH):
            # queue fillers: next chunk's projections first
            if c + 1 < CH:
                if c + 2 < CH:
                    fifo.append(lambda cc=c + 2: dma_x_chunk(cc))
                for kind, idx in _chunk_order():
                    fifo.append(lambda k=kind, i=idx, cc=c + 1: proj_group(cc, k, i))
            for h in range(HPC):
                attn_block(c, h)
                if h == 0 and c >= 1:
                    # row c-1's attoutT is fully normalized once block (c, h0)
                    # has flushed the pending aux -> stage-3 row c-1 is safe
                    for qt in range(4 * (c - 1), 4 * c):
                        for nch in range(4):
                            fifo.append(lambda q=qt, n=nch: st3_chunk(q, n))
            if c == CH - 2:
                # hold a few stage-3 fillers back for row 3's first block
                while len(fifo) > 4:
                    fifo.pop(0)()
            else:
                while fifo:
                    fifo.pop(0)()

        # Final stage-3 row with a deep pipeline: borrow the (now idle)
        # scores banks, start the first 5 chunks with h0-h2 partial chains
        # so the h3 matmuls land after the last normalization, and alternate
        # copies DVE/ACT.
        STG = 6
        staged_pools = [
            (avp, "av"), (scp, "sc"), (scp, "sc"), (pp, "pp"), (scp, "sc"), (pp, "pp"),
        ]

        def st3_tail(k, ps=None):
            qt, nch = 4 * (CH - 1) + k // 4, k % 4
            if ps is None:
                pool, tag = (scp, "sc") if k % 2 else (pp, "pp")
                ps = pool.tile([P, 512], f32, tag=tag, name="psy")
                h0 = 0
            else:
                h0 = HPC - 1
            for h in range(h0, HPC):
                nc.tensor.matmul(
                    ps,
                    attoutT_sb[:, h, qt * P : (qt + 1) * P],
                    wo_sb[:, h, nch * 512 : (nch + 1) * 512],
                    start=(h == 0),
                    stop=(h == HPC - 1),
                )
            yt = ysp.tile([P, 512], bf16, tag="yt", name="yt")
            if k % 2:
                nc.vector.tensor_copy(yt, ps)
            else:
                nc.scalar.copy(yt, ps)
            nc.sync.dma_start(out=y_r[:, qt, nch * 512 : (nch + 1) * 512], in_=yt)

        staged = []
        for k in range(STG):
            qt, nch = 4 * (CH - 1) + k // 4, k % 4
            pool, tag = staged_pools[k]
            ps = pool.tile([P, 512], f32, tag=tag, name="psy")
            for h in range(HPC - 1):
                nc.tensor.matmul(
                    ps,
                    attoutT_sb[:, h, qt * P : (qt + 1) * P],
                    wo_sb[:, h, nch * 512 : (nch + 1) * 512],
                    start=(h == 0),
                    stop=False,
                )
            staged.append(ps)
            if k == 0 and pending[0] is not None:
                emit_aux(pending[0])
                pending[0] = None
        for k in range(STG):
            st3_tail(k, ps=staged[k])
        for k in range(STG, 16):
            st3_tail(k)

    nc.compile()
    return nc


def _chunk_order():
    # q-heads first (reuse wq while wk/wv stream in at startup), then k0 so
    # the row's first scores have kT, then v, then the remaining k-heads
    order = [("q", h) for h in range(HPC)]
    order += [("k", 0)]
    order += [("v", t) for t in range(4)]
    order += [("k", h) for h in range(1, HPC)]
    return order


def _static_inputs():
    import ml_dtypes

    masks = np.zeros((4, P, 512), dtype=np.float32)
    kk = np.arange(P)[:, None]
    qq = np.arange(512)[None, :]
    for r in range(4):
        masks[r] = (P * r + kk <= qq).astype(np.float32)
    return masks.astype(ml_dtypes.bfloat16), np.eye(P, dtype=np.float32)


def make_in_maps(x, Wq, Wk, Wv, Wo):
    """Shard full inputs into 8 per-core input dicts (bf16)."""
    import ml_dtypes

    bf = ml_dtypes.bfloat16
    bm, identb = _static_inputs()
    scale = 1.0 / math.sqrt(d)
    in_maps = []
    for c in range(N_CORES):
        b, g = divmod(c, 4)
        hs = g * HPC * d  # 512*g: rows of Wq for this head group
        in_maps.append(
            {
                "xT": np.ascontiguousarray(x[b].T).astype(bf),
                "wq": (np.ascontiguousarray(Wq[hs : hs + 512, :].T) * np.float32(scale)).astype(bf),
                "wk": np.ascontiguousarray(Wk[hs : hs + 512, :].T).astype(bf),
                "wv": np.ascontiguousarray(Wv[hs : hs + 512, :].T).astype(bf),
                "wo": np.ascontiguousarray(Wo[:, hs : hs + 512].T).astype(bf),
                "bm": bm,
                "identb": identb,
            }
        )
    return in_maps


def combine_results(results):
    """results: list of 8 dicts with 'y' [S, D] bf16 partials -> full [B, S, D]."""
    y = np.zeros((B, S, D), dtype=np.float32)
    for c in range(N_CORES):
        b = c // 4
        y[b] += np.asarray(results[c]["y"], dtype=np.float32)
    return y


def _is_canonical_causal(attn_mask):
    m = np.asarray(attn_mask).reshape(S, S)
    iu = np.triu_indices(S, k=1)
    if not np.all(m[iu] <= -1e8):
        return False
    il = np.tril_indices(S, k=0)
    return np.all(m[il] == 0.0)


def _scores_safe(x, Wq, Wk):
    """Sampled bound on |scores| to make exp-without-max safe."""
    rng = np.random.default_rng(0)
    qi = rng.choice(S, 96, replace=False)
    ki = rng.choice(S, 384, replace=False)
    mx = 0.0
    for b in range(B):
        q = (x[b][qi] @ Wq.T) / math.sqrt(d)  # [96, D]
        k = x[b][ki] @ Wk.T  # [384, D]
        qh = q.reshape(96, H, d)
        kh = k.reshape(384, H, d)
        s = np.einsum("qhd,khd->hqk", qh, kh)
        mx = max(mx, float(np.abs(s).max()))
    return mx < 30.0


def _numpy_reference(x, attn_mask, Wq, Wk, Wv, Wo):
    out = np.zeros((B, S, D), dtype=np.float32)
    m = np.asarray(attn_mask, dtype=np.float32).reshape(S, S)
    for b in range(B):
        q = (x[b] @ Wq.T).reshape(S, H, d).transpose(1, 0, 2)
        k = (x[b] @ Wk.T).reshape(S, H, d).transpose(1, 0, 2)
        v = (x[b] @ Wv.T).reshape(S, H, d).transpose(1, 0, 2)
        q = q / np.float32(math.sqrt(d))
        att_out = np.zeros((H, S, d), dtype=np.float32)
        for h in range(H):
            s = q[h] @ k[h].T + m
            s = s - s.max(axis=-1, keepdims=True)
            p = np.exp(s)
            p /= p.sum(axis=-1, keepdims=True)
            att_out[h] = p @ v[h]
        out[b] = att_out.transpose(1, 0, 2).reshape(S, D) @ Wo.T
    return out


def kernel(x, attn_mask, Wq, Wk, Wv, Wo):
    x = np.asarray(x, dtype=np.float32)
    Wq = np.asarray(Wq, dtype=np.float32)
    Wk = np.asarray(Wk, dtype=np.float32)
    Wv = np.asarray(Wv, dtype=np.float32)
    Wo = np.asarray(Wo, dtype=np.float32)

    if not _is_canonical_causal(attn_mask) or not _scores_safe(x, Wq, Wk):
        return _numpy_reference(x, attn_mask, Wq, Wk, Wv, Wo)

    from concourse.bass_utils import run_bass_kernel_spmd

    if "nc" not in _CACHE:
        _CACHE["nc"] = _build_module()
    nc = _CACHE["nc"]

    in_maps = make_in_maps(x, Wq, Wk, Wv, Wo)
    res = run_bass_kernel_spmd(nc, in_maps, core_ids=list(range(N_CORES)))
    return combine_results(res.results)


# revision 49
# speedup vs baseline: 1.2589x; 1.0125x over previous
"""Causal self-attention (B=2, S=2048, D=2048, H=16) on 8 TRN2 NeuronCores.

Sharding: tensor-parallel over heads x data-parallel over batch.
Core c = b*4 + g handles batch b and heads 4g..4g+3 (head_dim=128).

Single-NEFF SPMD design (all-bf16 matmuls, wavefront schedule):
  - x is streamed from DRAM once (bf16, transposed layout xT [D, S]), in 4
    chunks of 512 tokens.  Per chunk: q/k projections per head (qT/kT [d, S]
    bf16) and v projection (natural [tok, d] bf16), all bf16 matmuls with
    fp32 PSUM accumulation.
  - After chunk c, attention "row" c runs for all 4 heads: transposed scores
    sT[j] = k_j @ qT_block (f32 PSUM, trimmed to the causal triangle at
    128-col granularity), exp on ACT -> probs bf16, binary causal mask on
    diagonal-strip tiles (DVE), then avT[d, 512q] += v_j.T @ probs_j on PE.
    Softmax denominators come from near-free ap=1 matmuls
    (probs_j[:,qt].T @ ones).  Normalization: reciprocal (DVE), PE-transpose
    of the [128,4] rec vector, partition-broadcast (GPSIMD) to a [128,512]
    scale tile, one DVE multiply into attoutT bf16.
  - Stage-3 output projection chunks (y[qt] = sum_h attoutT_h.T @ Wo_h) are
    interleaved as PE "filler" work one row behind attention, so the ACT exp
    stream never stalls the PE.  y is written bf16; the host accumulates the
    4 per-core partials per batch in fp32.

Softmax skips the max-subtraction (scores are O(5..30) for the expected
input distribution; a host-side sampling guard falls back to a numpy
reference if scores could overflow exp, or if the mask is not the canonical
causal mask).
"""

import math
from contextlib import ExitStack

import numpy as np

B = 2
S = 2048
D = 2048
H = 16
HPC = 4  # heads per core
d = 128  # head dim
N_CORES = 8
P = 128
DK = D // P  # 16 contraction tiles
ST = S // P  # 16 token tiles
CH = S // 512  # 4 chunks of 512 tokens
PROBS_BUFS = 20

_CACHE = {}


def _build_module():
    import concourse.mybir as mybir
    import concourse.tile as tile
    from concourse import bacc

    f32 = mybir.dt.float32
    f32r = mybir.dt.float32r
    bf16 = mybir.dt.bfloat16
    Exp = mybir.ActivationFunctionType.Exp

    nc = bacc.Bacc("TRN2", target_bir_lowering=False, debug=False)

    xT = nc.dram_tensor("xT", [D, S], bf16, kind="ExternalInput")
    wq = nc.dram_tensor("wq", [D, HPC * d], bf16, kind="ExternalInput")
    wk = nc.dram_tensor("wk", [D, HPC * d], bf16, kind="ExternalInput")
    wv = nc.dram_tensor("wv", [D, HPC * d], bf16, kind="ExternalInput")
    wo = nc.dram_tensor("wo", [HPC * d, D], bf16, kind="ExternalInput")
    bm = nc.dram_tensor("bm", [4, P, 512], bf16, kind="ExternalInput")
    identb = nc.dram_tensor("identb", [P, P], f32, kind="ExternalInput")
    y = nc.dram_tensor("y", [S, D], bf16, kind="ExternalOutput")

    xT_r = xT.ap().rearrange("(t p) s -> p t s", p=P)
    wq_r = wq.ap().rearrange("(t p) m -> p t m", p=P)
    wk_r = wk.ap().rearrange("(t p) m -> p t m", p=P)
    wv_r = wv.ap().rearrange("(t p) m -> p t m", p=P)
    wo_r = wo.ap().rearrange("(t p) n -> p t n", p=P)
    y_r = y.ap().rearrange("(t p) n -> p t n", p=P)

    with tile.TileContext(nc) as tc, ExitStack() as top:
        consts = top.enter_context(tc.tile_pool(name="consts", bufs=1))
        wpool = top.enter_context(tc.tile_pool(name="wpool", bufs=1))
        wop = top.enter_context(tc.tile_pool(name="wop", bufs=1))
        xp = top.enter_context(tc.tile_pool(name="xp", bufs=2))
        qkp = top.enter_context(tc.tile_pool(name="qkp", bufs=1))
        vp = top.enter_context(tc.tile_pool(name="vp", bufs=1))
        aop = top.enter_context(tc.tile_pool(name="aop", bufs=1))
        probp = top.enter_context(tc.tile_pool(name="probp", bufs=PROBS_BUFS))
        smallp = top.enter_context(tc.tile_pool(name="smallp", bufs=3))
        ysp = top.enter_context(tc.tile_pool(name="ysp", bufs=8))
        # PSUM: pp(2) + sc(3) + av(2) + scr(1) = 8 banks
        pp = top.enter_context(tc.tile_pool(name="pp", bufs=2, space="PSUM"))
        scp = top.enter_context(tc.tile_pool(name="scp", bufs=3, space="PSUM"))
        avp = top.enter_context(tc.tile_pool(name="avp", bufs=2, space="PSUM"))
        scrp = top.enter_context(tc.tile_pool(name="scrp", bufs=1, space="PSUM"))

        bm_sb = consts.tile([P, 4, 512], bf16, tag="bm")
        id_sb = consts.tile([P, P], f32, tag="identb")
        ones_col = consts.tile([P, 1], bf16, tag="ones")
        wq_sb = wpool.tile([P, DK, HPC * d], bf16, tag="wq")
        wk_sb = wpool.tile([P, DK, HPC * d], bf16, tag="wk")
        wv_sb = wpool.tile([P, DK, HPC * d], bf16, tag="wv")
        wo_sb = wop.tile([P, HPC, D], bf16, tag="wo")
        qT_sb = qkp.tile([P, HPC, S], bf16, tag="qT")
        kT_sb = qkp.tile([P, HPC, S], bf16, tag="kT")
        v_sb = vp.tile([P, ST, HPC, d], bf16, tag="v")
        attoutT_sb = aop.tile([P, HPC, S], bf16, tag="attoutT")
        scr = scrp.tile([P, 512], f32, tag="scr")  # softmax denominators, cols 0:4

        nc.gpsimd.memset(ones_col, 1.0)
        warm = consts.tile([P, 512], bf16, tag="warm")
        nc.gpsimd.memset(warm, 0.0)
        wps = scp.tile([P, 512], f32, tag="sc", name="warmps")
        for wi in range(8):
            nc.tensor.matmul(
                warm if False else wps,
                warm[:, 0:P],
                warm,
                start=(wi == 0),
                stop=(wi == 7),
            )

        # ---- initial DMAs (SP ring, priority order) ----
        xc_tiles = [None] * CH

        def dma_x_chunk(c):
            t = xp.tile([P, DK, 512], bf16, tag="xc", name=f"xc{c}")
            for p4 in range(4):
                nc.sync.dma_start(
                    out=t[:, 4 * p4 : 4 * p4 + 4, :],
                    in_=xT_r[:, 4 * p4 : 4 * p4 + 4, c * 512 : (c + 1) * 512],
                )
            xc_tiles[c] = t

        xc0 = xp.tile([P, DK, 512], bf16, tag="xc", name="xc0")
        xc_tiles[0] = xc0
        # interleave wq/x0 in small pieces: the tile-major chunk-0 schedule
        # below consumes the stream at line rate
        bounds = [0, 1, 2, 4, 6, 8, 10, 12, 14, 16]
        for a, b in zip(bounds[:-1], bounds[1:]):
            nc.sync.dma_start(out=wq_sb[:, a:b, :], in_=wq_r[:, a:b, :])
            nc.sync.dma_start(out=xc0[:, a:b, :], in_=xT_r[:, a:b, 0:512])
        for w_sb, w_r in ((wk_sb, wk_r), (wv_sb, wv_r)):
            for p2 in range(8):
                nc.sync.dma_start(
                    out=w_sb[:, 2 * p2 : 2 * p2 + 2, :],
                    in_=w_r[:, 2 * p2 : 2 * p2 + 2, :],
                )
        dma_x_chunk(1)
        nc.sync.dma_start(out=bm_sb, in_=bm.ap().rearrange("r p m -> p r m"))
        nc.sync.dma_start(out=id_sb, in_=identb.ap())
        for p4 in range(4):
            nc.sync.dma_start(out=wo_sb[:, p4, :], in_=wo_r[:, p4, :])

        # ---- work units ----
        def proj_group(c, kind, idx):
            """kind: 'q'/'k' with idx=head, 'v' with idx=token-subtile."""
            xc = xc_tiles[c]
            ps = pp.tile([P, 512], f32, tag="pp", name="psp")
            if kind == "v":
                for kk in range(DK):
                    nc.tensor.matmul(
                        ps,
                        xc[:, kk, idx * P : (idx + 1) * P],
                        wv_sb[:, kk, :],
                        start=(kk == 0),
                        stop=(kk == DK - 1),
                    )
                nc.vector.tensor_copy(
                    v_sb[:, c * 4 + idx, :, :],
                    ps.rearrange("p (h e) -> p h e", h=HPC),
                )
            else:
                w_sb = wq_sb if kind == "q" else wk_sb
                dest = qT_sb if kind == "q" else kT_sb
                for kk in range(DK):
                    nc.tensor.matmul(
                        ps,
                        w_sb[:, kk, idx * d : (idx + 1) * d],
                        xc[:, kk, :],
                        start=(kk == 0),
                        stop=(kk == DK - 1),
                    )
                nc.vector.tensor_copy(dest[:, idx, c * 512 : (c + 1) * 512], ps)

        st3_state = {"mode": "dve", "alt": 0}

        def st3_chunk(qt, nch):
            ps = pp.tile([P, 512], f32, tag="pp", name="psy")
            for h in range(HPC):
                nc.tensor.matmul(
                    ps,
                    attoutT_sb[:, h, qt * P : (qt + 1) * P],
                    wo_sb[:, h, nch * 512 : (nch + 1) * 512],
                    start=(h == 0),
                    stop=(h == HPC - 1),
                )
            yt = ysp.tile([P, 512], bf16, tag="yt", name="yt")
            st3_state["alt"] += 1
            if st3_state["mode"] == "mix" and st3_state["alt"] % 2 == 0:
                nc.scalar.copy(yt, ps)
            else:
                nc.vector.tensor_copy(yt, ps)
            nc.sync.dma_start(out=y_r[:, qt, nch * 512 : (nch + 1) * 512], in_=yt)

        # filler fifo of (kind, thunk)
        fifo = []

        def filler(n=1):
            for _ in range(n):
                if fifo:
                    fifo.pop(0)[1]()

        def emit_aux(blk, fast=False):
            """Normalization tail of a finished block: one PE transpose of the
            [128,4] rec vector, DVE copy to SBUF, a tiny SBUF->SBUF DMA that
            flattens [4,128] onto partition 0, partition-broadcast (GPSIMD) to
            [128,512], then one DVE multiply applies 1/den along the free (q)
            axis of avT."""
            c, h, avT, rec = blk
            recT_ps = scp.tile([P, 512], f32, tag="sc", name="recT_ps")
            recT = smallp.tile([1, 512], bf16, tag="recT", name="recT")
            if fast:
                # latency-optimized tail for the last block: four transposes
                # put rec flat on partition 0 directly (no DMA round-trip)
                for qt in range(4):
                    nc.tensor.matmul(
                        recT_ps[0:1, qt * P : (qt + 1) * P],
                        rec[:, qt : qt + 1],
                        id_sb,
                        is_transpose=True,
                        start=(qt == 0),
                        stop=(qt == 3),
                    )
                nc.vector.tensor_copy(recT, recT_ps[0:1, 0:512])
            else:
                nc.tensor.transpose(recT_ps[0:4, 0:P], rec, id_sb)
                recT4 = smallp.tile([4, P], bf16, tag="recT4", name="recT4")
                nc.vector.tensor_copy(recT4, recT_ps[0:4, 0:P])
                nc.scalar.dma_start(out=recT, in_=recT4)
            filler()
            rb = smallp.tile([P, 512], bf16, tag="rb", name="rb")
            nc.gpsimd.partition_broadcast(rb, recT)
            nc.vector.tensor_mul(
                attoutT_sb[:, h, c * 512 : (c + 1) * 512], avT, rb
            )

        pending = [None]  # block awaiting its normalization tail

        def attn_block(c, h):
            NK = 4 * c + 4
            step = 4
            probs = []
            for j in range(NK):
                r = j - 4 * c
                lo = P * r if r > 0 else 0
                sc = scp.tile([P, 512], f32, tag="sc", name="sc")
                nc.tensor.matmul(
                    sc[:, lo:512],
                    kT_sb[:, h, j * P : (j + 1) * P],
                    qT_sb[:, h, c * 512 + lo : (c + 1) * 512],
                    start=True,
                    stop=True,
                )
                pj = probp.tile([P, 512], bf16, tag="probs", name="pj")
                nc.scalar.activation(out=pj[:, lo:512], in_=sc[:, lo:512], func=Exp)
                if r >= 0:
                    # trimmed: no consumer ever reads pj[:, :lo]
                    nc.vector.tensor_mul(
                        pj[:, lo:512], pj[:, lo:512], bm_sb[:, r, lo:512]
                    )
                probs.append(pj)
                if j % step == step - 1:
                    filler()
                if j == 7 and pending[0] is not None:
                    # flush early: queues the aux's DVE work ahead of this
                    # block's later masks, shortening the chain's latency
                    emit_aux(pending[0])
                    pending[0] = None
            if pending[0] is not None:
                emit_aux(pending[0])
                pending[0] = None
            avT = avp.tile([P, 512], f32, tag="av", name="avT")
            for j in range(NK):
                r = j - 4 * c
                lo = P * r if r > 0 else 0
                nc.tensor.matmul(
                    avT[:, lo:512],
                    v_sb[:, j, h, :],
                    probs[j][:, lo:512],
                    start=(j == 0),
                    stop=(j == NK - 1),
                )
                for qt in range(max(r, 0), 4):
                    # all four chains share one PSUM zero region (the whole
                    # bank): only the first matmul starts it, only the last
                    # one stops it
                    nc.tensor.matmul(
                        scr[:, qt : qt + 1],
                        probs[j][:, qt * P : (qt + 1) * P],
                        ones_col,
                        start=(j == 0 and qt == max(r, 0)),
                        stop=(j == NK - 1 and qt == 3),
                    )
                if j % step == step - 1:
                    filler()
            rec = smallp.tile([P, 4], f32, tag="rec", name="rec")
            nc.vector.reciprocal(rec, scr[:, 0:4])
            pending[0] = (c, h, avT, rec)

        # ---- main wavefront ----
        # chunk 0 runs tile-major: four open PSUM chains consume each weight/x
        # tile as it lands, so the PE tracks the startup DMA stream
        for kind in ("q", "k", "v"):
            chains = [
                pp.tile([P, 512], f32, tag="pp", name="c0ps"),
                pp.tile([P, 512], f32, tag="pp", name="c0ps"),
                scp.tile([P, 512], f32, tag="sc", name="c0ps"),
                scp.tile([P, 512], f32, tag="sc", name="c0ps"),
            ]
            w_sb = {"q": wq_sb, "k": wk_sb, "v": wv_sb}[kind]
            for t in range(DK):
                for i in range(4):
                    if kind == "v":
                        nc.tensor.matmul(
                            chains[i],
                            xc0[:, t, i * P : (i + 1) * P],
                            wv_sb[:, t, :],
                            start=(t == 0),
                            stop=(t == DK - 1),
                        )
                    else:
                        nc.tensor.matmul(
                            chains[i],
                            w_sb[:, t, i * d : (i + 1) * d],
                            xc0[:, t, :],
                            start=(t == 0),
                            stop=(t == DK - 1),
                        )
            for i in range(4):
                if kind == "v":
                    nc.vector.tensor_copy(
                        v_sb[:, i, :, :],
                        chains[i].rearrange("p (h e) -> p h e", h=HPC),
                    )
                else:
                    dest = qT_sb if kind == "q" else kT_sb
                    nc.vector.tensor_copy(dest[:, i, 0:512], chains[i])

        deferred = []
        for c in range(CH):
            # queue fillers: next chunk's projections first
            if c + 1 < CH:
                if c + 2 < CH:
                    fifo.append(("dma", lambda cc=c + 2: dma_x_chunk(cc)))
                for kind, idx in _chunk_order():
                    fifo.append(
                        ("proj", lambda k=kind, i=idx, cc=c + 1: proj_group(cc, k, i))
                    )
            if c == CH - 1:
                # chunks deferred from row 2: ready filler for row 3's first
                # block, which otherwise has nothing to hide exp latency with
                fifo.extend(deferred)
                deferred = []
            for h in range(HPC):
                attn_block(c, h)
                if h == 0 and c >= 1:
                    # row c-1's attoutT is fully normalized once block (c, h0)
                    # has flushed the pending aux -> stage-3 row c-1 is safe
                    thunks = [
                        ("st3", lambda q=qt, n=nch: st3_chunk(q, n))
                        for qt in range(4 * (c - 1), 4 * c)
                        for nch in range(4)
                    ]
                    if c == CH - 2:
                        deferred = thunks[-8:]
                        thunks = thunks[:-8]
                    fifo.extend(thunks)
            if c == CH - 2:
                # drain only what row 3 depends on (chunk-3 projections and
                # x DMAs); leftover stage-3 chunks stay as row-3 fillers
                while fifo and fifo[0][0] != "st3":
                    fifo.pop(0)[1]()
            else:
                while fifo:
                    fifo.pop(0)[1]()

        # Final stage-3 row with a deep pipeline: borrow the (now idle)
        # scores banks, start the first 5 chunks with h0-h2 partial chains
        # so the h3 matmuls land after the last normalization, and alternate
        # copies DVE/ACT.
        STG = 6
        staged_pools = [
            (avp, "av"), (scp, "sc"), (scp, "sc"), (pp, "pp"), (scp, "sc"), (pp, "pp"),
        ]

        def st3_tail(k, ps=None):
            qt, nch = 4 * (CH - 1) + k // 4, k % 4
            if ps is None:
                pool, tag = (scp, "sc") if k % 2 else (pp, "pp")
                ps = pool.tile([P, 512], f32, tag=tag, name="psy")
                h0 = 0
            else:
                h0 = HPC - 1
            for h in range(h0, HPC):
                nc.tensor.matmul(
                    ps,
                    attoutT_sb[:, h, qt * P : (qt + 1) * P],
                    wo_sb[:, h, nch * 512 : (nch + 1) * 512],
                    start=(h == 0),
                    stop=(h == HPC - 1),
                )
            yt = ysp.tile([P, 512], bf16, tag="yt", name="yt")
            if k % 2:
                nc.vector.tensor_copy(yt, ps)
            else:
                nc.scalar.copy(yt, ps)
            nc.sync.dma_start(out=y_r[:, qt, nch * 512 : (nch + 1) * 512], in_=yt)

        staged = []
        for k in range(STG):
            qt, nch = 4 * (CH - 1) + k // 4, k % 4
            pool, tag = staged_pools[k]
            ps = pool.tile([P, 512], f32, tag=tag, name="psy")
            for h in range(HPC - 1):
                nc.tensor.matmul(
                    ps,
                    attoutT_sb[:, h, qt * P : (qt + 1) * P],
                    wo_sb[:, h, nch * 512 : (nch + 1) * 512],
                    start=(h == 0),
                    stop=False,
                )
            staged.append(ps)
            if k == 0 and pending[0] is not None:
                emit_aux(pending[0], fast=True)
                pending[0] = None
        for k in range(STG):
            st3_tail(k, ps=staged[k])
        for k in range(STG, 16):
            st3_tail(k)

    nc.compile()
    return nc


def _chunk_order():
    # q-heads first (reuse wq while wk/wv stream in at startup), then k0 so
    # the row's first scores have kT, then v, then the remaining k-heads
    order = [("q", h) for h in range(HPC)]
    order += [("k", 0)]
    order += [("v", t) for t in range(4)]
    order += [("k", h) for h in range(1, HPC)]
    return order


def _static_inputs():
    import ml_dtypes

    masks = np.zeros((4, P, 512), dtype=np.float32)
    kk = np.arange(P)[:, None]
    qq = np.arange(512)[None, :]
    for r in range(4):
        masks[r] = (P * r + kk <= qq).astype(np.float32)
    return masks.astype(ml_dtypes.bfloat16), np.eye(P, dtype=np.float32)


def make_in_maps(x, Wq, Wk, Wv, Wo):
    """Shard full inputs into 8 per-core input dicts (bf16)."""
    import ml_dtypes

    bf = ml_dtypes.bfloat16
    bm, identb = _static_inputs()
    scale = 1.0 / math.sqrt(d)
    in_maps = []
    for c in range(N_CORES):
        b, g = divmod(c, 4)
        hs = g * HPC * d  # 512*g: rows of Wq for this head group
        in_maps.append(
            {
                "xT": np.ascontiguousarray(x[b].T).astype(bf),
                "wq": (np.ascontiguousarray(Wq[hs : hs + 512, :].T) * np.float32(scale)).astype(bf),
                "wk": np.ascontiguousarray(Wk[hs : hs + 512, :].T).astype(bf),
                "wv": np.ascontiguousarray(Wv[hs : hs + 512, :].T).astype(bf),
                "wo": np.ascontiguousarray(Wo[:, hs : hs + 512].T).astype(bf),
                "bm": bm,
                "identb": identb,
            }
        )
    return in_maps


def combine_results(results):
    """results: list of 8 dicts with 'y' [S, D] bf16 partials -> full [B, S, D]."""
    y = np.zeros((B, S, D), dtype=np.float32)
    for c in range(N_CORES):
        b = c // 4
        y[b] += np.asarray(results[c]["y"], dtype=np.float32)
    return y


def _is_canonical_causal(attn_mask):
    m = np.asarray(attn_mask).reshape(S, S)
    iu = np.triu_indices(S, k=1)
    if not np.all(m[iu] <= -1e8):
        return False
    il = np.tril_indices(S, k=0)
    return np.all(m[il] == 0.0)


def _scores_safe(x, Wq, Wk):
    """Sampled bound on |scores| to make exp-without-max safe."""
    rng = np.random.default_rng(0)
    qi = rng.choice(S, 96, replace=False)
    ki = rng.choice(S, 384, replace=False)
    mx = 0.0
    for b in range(B):
        q = (x[b][qi] @ Wq.T) / math.sqrt(d)  # [96, D]
        k = x[b][ki] @ Wk.T  # [384, D]
        qh = q.reshape(96, H, d)
        kh = k.reshape(384, H, d)
        s = np.einsum("qhd,khd->hqk", qh, kh)
        mx = max(mx, float(np.abs(s).max()))
    return mx < 30.0


def _numpy_reference(x, attn_mask, Wq, Wk, Wv, Wo):
    out = np.zeros((B, S, D), dtype=np.float32)
    m = np.asarray(attn_mask, dtype=np.float32).reshape(S, S)
    for b in range(B):
        q = (x[b] @ Wq.T).reshape(S, H, d).transpose(1, 0, 2)
        k = (x[b] @ Wk.T).reshape(S, H, d).transpose(1, 0, 2)
        v = (x[b] @ Wv.T).reshape(S, H, d).transpose(1, 0, 2)
        q = q / np.float32(math.sqrt(d))
        att_out = np.zeros((H, S, d), dtype=np.float32)
        for h in range(H):
            s = q[h] @ k[h].T + m
            s = s - s.max(axis=-1, keepdims=True)
            p = np.exp(s)
            p /= p.sum(axis=-1, keepdims=True)
            att_out[h] = p @ v[h]
        out[b] = att_out.transpose(1, 0, 2).reshape(S, D) @ Wo.T
    return out


def kernel(x, attn_mask, Wq, Wk, Wv, Wo):
    x = np.asarray(x, dtype=np.float32)
    Wq = np.asarray(Wq, dtype=np.float32)
    Wk = np.asarray(Wk, dtype=np.float32)
    Wv = np.asarray(Wv, dtype=np.float32)
    Wo = np.asarray(Wo, dtype=np.float32)

    if not _is_canonical_causal(attn_mask) or not _scores_safe(x, Wq, Wk):
        return _numpy_reference(x, attn_mask, Wq, Wk, Wv, Wo)

    from concourse.bass_utils import run_bass_kernel_spmd

    if "nc" not in _CACHE:
        _CACHE["nc"] = _build_module()
    nc = _CACHE["nc"]

    in_maps = make_in_maps(x, Wq, Wk, Wv, Wo)
    res = run_bass_kernel_spmd(nc, in_maps, core_ids=list(range(N_CORES)))
    return combine_results(res.results)


# revision 57
# speedup vs baseline: 1.2606x; 1.0013x over previous
"""Causal self-attention (B=2, S=2048, D=2048, H=16) on 8 TRN2 NeuronCores.

Sharding: tensor-parallel over heads x data-parallel over batch.
Core c = b*4 + g handles batch b and heads 4g..4g+3 (head_dim=128).

Single-NEFF SPMD design (all-bf16 matmuls, wavefront schedule):
  - x is streamed from DRAM once (bf16, transposed layout xT [D, S]), in 4
    chunks of 512 tokens.  Per chunk: q/k projections per head (qT/kT [d, S]
    bf16) and v projection (natural [tok, d] bf16), all bf16 matmuls with
    fp32 PSUM accumulation.
  - After chunk c, attention "row" c runs for all 4 heads: transposed scores
    sT[j] = k_j @ qT_block (f32 PSUM, trimmed to the causal triangle at
    128-col granularity), exp on ACT -> probs bf16, binary causal mask on
    diagonal-strip tiles (DVE), then avT[d, 512q] += v_j.T @ probs_j on PE.
    Softmax denominators come from near-free ap=1 matmuls
    (probs_j[:,qt].T @ ones).  Normalization: reciprocal (DVE), PE-transpose
    of the [128,4] rec vector, partition-broadcast (GPSIMD) to a [128,512]
    scale tile, one DVE multiply into attoutT bf16.
  - Stage-3 output projection chunks (y[qt] = sum_h attoutT_h.T @ Wo_h) are
    interleaved as PE "filler" work one row behind attention, so the ACT exp
    stream never stalls the PE.  y is written bf16; the host accumulates the
    4 per-core partials per batch in fp32.

Softmax skips the max-subtraction (scores are O(5..30) for the expected
input distribution; a host-side sampling guard falls back to a numpy
reference if scores could overflow exp, or if the mask is not the canonical
causal mask).
"""

import math
from contextlib import ExitStack

import numpy as np

B = 2
S = 2048
D = 2048
H = 16
HPC = 4  # heads per core
d = 128  # head dim
N_CORES = 8
P = 128
DK = D // P  # 16 contraction tiles
ST = S // P  # 16 token tiles
CH = S // 512  # 4 chunks of 512 tokens
PROBS_BUFS = 20

_CACHE = {}


def _build_module():
    import concourse.mybir as mybir
    import concourse.tile as tile
    from concourse import bacc

    f32 = mybir.dt.float32
    f32r = mybir.dt.float32r
    bf16 = mybir.dt.bfloat16
    Exp = mybir.ActivationFunctionType.Exp

    nc = bacc.Bacc("TRN2", target_bir_lowering=False, debug=False)

    xT = nc.dram_tensor("xT", [D, S], bf16, kind="ExternalInput")
    wq = nc.dram_tensor("wq", [D, HPC * d], bf16, kind="ExternalInput")
    wk = nc.dram_tensor("wk", [D, HPC * d], bf16, kind="ExternalInput")
    wv = nc.dram_tensor("wv", [D, HPC * d], bf16, kind="ExternalInput")
    wo = nc.dram_tensor("wo", [HPC * d, D], bf16, kind="ExternalInput")
    bm = nc.dram_tensor("bm", [4, P, 512], bf16, kind="ExternalInput")
    identb = nc.dram_tensor("identb", [P, P], f32, kind="ExternalInput")
    y = nc.dram_tensor("y", [S, D], bf16, kind="ExternalOutput")

    xT_r = xT.ap().rearrange("(t p) s -> p t s", p=P)
    wq_r = wq.ap().rearrange("(t p) m -> p t m", p=P)
    wk_r = wk.ap().rearrange("(t p) m -> p t m", p=P)
    wv_r = wv.ap().rearrange("(t p) m -> p t m", p=P)
    wo_r = wo.ap().rearrange("(t p) n -> p t n", p=P)
    y_r = y.ap().rearrange("(t p) n -> p t n", p=P)

    with tile.TileContext(nc) as tc, ExitStack() as top:
        consts = top.enter_context(tc.tile_pool(name="consts", bufs=1))
        wpool = top.enter_context(tc.tile_pool(name="wpool", bufs=1))
        wop = top.enter_context(tc.tile_pool(name="wop", bufs=1))
        xp = top.enter_context(tc.tile_pool(name="xp", bufs=2))
        qkp = top.enter_context(tc.tile_pool(name="qkp", bufs=1))
        vp = top.enter_context(tc.tile_pool(name="vp", bufs=1))
        aop = top.enter_context(tc.tile_pool(name="aop", bufs=1))
        probp = top.enter_context(tc.tile_pool(name="probp", bufs=PROBS_BUFS))
        smallp = top.enter_context(tc.tile_pool(name="smallp", bufs=3))
        ysp = top.enter_context(tc.tile_pool(name="ysp", bufs=6))
        # PSUM: pp(2) + sc(3) + av(2) + scr(1) = 8 banks
        pp = top.enter_context(tc.tile_pool(name="pp", bufs=2, space="PSUM"))
        scp = top.enter_context(tc.tile_pool(name="scp", bufs=3, space="PSUM"))
        avp = top.enter_context(tc.tile_pool(name="avp", bufs=2, space="PSUM"))
        scrp = top.enter_context(tc.tile_pool(name="scrp", bufs=1, space="PSUM"))

        bm_sb = consts.tile([P, 4, 512], bf16, tag="bm")
        id_sb = consts.tile([P, P], f32, tag="identb")
        ones_col = consts.tile([P, 1], bf16, tag="ones")
        wq_sb = wpool.tile([P, DK, HPC * d], bf16, tag="wq")
        wk_sb = wpool.tile([P, DK, HPC * d], bf16, tag="wk")
        wv_sb = wpool.tile([P, DK, HPC * d], bf16, tag="wv")
        wo_sb = wop.tile([P, HPC, D], bf16, tag="wo")
        qT_sb = qkp.tile([P, HPC, S], bf16, tag="qT")
        kT_sb = qkp.tile([P, HPC, S], bf16, tag="kT")
        v_sb = vp.tile([P, ST, HPC, d], bf16, tag="v")
        attoutT_sb = aop.tile([P, HPC, S], bf16, tag="attoutT")
        scr = scrp.tile([P, 512], f32, tag="scr")  # softmax denominators, cols 0:4

        nc.gpsimd.memset(ones_col, 1.0)
        warm = consts.tile([P, 512], bf16, tag="warm")
        nc.gpsimd.memset(warm, 0.0)
        wps = scp.tile([P, 512], f32, tag="sc", name="warmps")
        for wi in range(8):
            nc.tensor.matmul(
                warm if False else wps,
                warm[:, 0:P],
                warm,
                start=(wi == 0),
                stop=(wi == 7),
            )

        # ---- initial DMAs (SP ring, priority order) ----
        xc_tiles = [None] * CH

        def dma_x_chunk(c):
            t = xp.tile([P, DK, 512], bf16, tag="xc", name=f"xc{c}")
            for p4 in range(4):
                nc.sync.dma_start(
                    out=t[:, 4 * p4 : 4 * p4 + 4, :],
                    in_=xT_r[:, 4 * p4 : 4 * p4 + 4, c * 512 : (c + 1) * 512],
                )
            xc_tiles[c] = t

        xc0 = xp.tile([P, DK, 512], bf16, tag="xc", name="xc0")
        xc_tiles[0] = xc0
        # interleave wq/x0 in small pieces: the tile-major chunk-0 schedule
        # below consumes the stream at line rate
        bounds = [0, 1, 2, 4, 6, 8, 10, 12, 14, 16]
        for a, b in zip(bounds[:-1], bounds[1:]):
            nc.sync.dma_start(out=wq_sb[:, a:b, :], in_=wq_r[:, a:b, :])
            nc.sync.dma_start(out=xc0[:, a:b, :], in_=xT_r[:, a:b, 0:512])
        for w_sb, w_r in ((wk_sb, wk_r), (wv_sb, wv_r)):
            for p2 in range(8):
                nc.sync.dma_start(
                    out=w_sb[:, 2 * p2 : 2 * p2 + 2, :],
                    in_=w_r[:, 2 * p2 : 2 * p2 + 2, :],
                )
        dma_x_chunk(1)
        nc.sync.dma_start(out=bm_sb, in_=bm.ap().rearrange("r p m -> p r m"))
        nc.sync.dma_start(out=id_sb, in_=identb.ap())
        for p4 in range(4):
            nc.sync.dma_start(out=wo_sb[:, p4, :], in_=wo_r[:, p4, :])

        # ---- work units ----
        def proj_group(c, kind, idx):
            """kind: 'q'/'k' with idx=head, 'v' with idx=token-subtile."""
            xc = xc_tiles[c]
            ps = pp.tile([P, 512], f32, tag="pp", name="psp")
            if kind == "v":
                for kk in range(DK):
                    nc.tensor.matmul(
                        ps,
                        xc[:, kk, idx * P : (idx + 1) * P],
                        wv_sb[:, kk, :],
                        start=(kk == 0),
                        stop=(kk == DK - 1),
                    )
                nc.vector.tensor_copy(
                    v_sb[:, c * 4 + idx, :, :],
                    ps.rearrange("p (h e) -> p h e", h=HPC),
                )
            else:
                w_sb = wq_sb if kind == "q" else wk_sb
                dest = qT_sb if kind == "q" else kT_sb
                for kk in range(DK):
                    nc.tensor.matmul(
                        ps,
                        w_sb[:, kk, idx * d : (idx + 1) * d],
                        xc[:, kk, :],
                        start=(kk == 0),
                        stop=(kk == DK - 1),
                    )
                nc.vector.tensor_copy(dest[:, idx, c * 512 : (c + 1) * 512], ps)

        st3_state = {"mode": "dve", "alt": 0}

        def st3_chunk(qt, nch):
            ps = pp.tile([P, 512], f32, tag="pp", name="psy")
            for h in range(HPC):
                nc.tensor.matmul(
                    ps,
                    attoutT_sb[:, h, qt * P : (qt + 1) * P],
                    wo_sb[:, h, nch * 512 : (nch + 1) * 512],
                    start=(h == 0),
                    stop=(h == HPC - 1),
                )
            yt = ysp.tile([P, 512], bf16, tag="yt", name="yt")
            st3_state["alt"] += 1
            if st3_state["mode"] == "mix" and st3_state["alt"] % 2 == 0:
                nc.scalar.copy(yt, ps)
            else:
                nc.vector.tensor_copy(yt, ps)
            nc.sync.dma_start(out=y_r[:, qt, nch * 512 : (nch + 1) * 512], in_=yt)

        # filler fifo of (kind, thunk)
        fifo = []

        def filler(n=1):
            for _ in range(n):
                if fifo:
                    fifo.pop(0)[1]()

        def emit_aux(blk, fast=False):
            """Normalization tail of a finished block: one PE transpose of the
            [128,4] rec vector, DVE copy to SBUF, a tiny SBUF->SBUF DMA that
            flattens [4,128] onto partition 0, partition-broadcast (GPSIMD) to
            [128,512], then one DVE multiply applies 1/den along the free (q)
            axis of avT."""
            c, h, avT, rec = blk
            recT_ps = scp.tile([P, 512], f32, tag="sc", name="recT_ps")
            recT = smallp.tile([1, 512], bf16, tag="recT", name="recT")
            if fast:
                # latency-optimized tail for the last block: four transposes
                # put rec flat on partition 0 directly (no DMA round-trip)
                for qt in range(4):
                    nc.tensor.matmul(
                        recT_ps[0:1, qt * P : (qt + 1) * P],
                        rec[:, qt : qt + 1],
                        id_sb,
                        is_transpose=True,
                        start=(qt == 0),
                        stop=(qt == 3),
                    )
                nc.vector.tensor_copy(recT, recT_ps[0:1, 0:512])
            else:
                nc.tensor.transpose(recT_ps[0:4, 0:P], rec, id_sb)
                recT4 = smallp.tile([4, P], bf16, tag="recT4", name="recT4")
                nc.vector.tensor_copy(recT4, recT_ps[0:4, 0:P])
                nc.scalar.dma_start(out=recT, in_=recT4)
            filler()
            rb = smallp.tile([P, 512], bf16, tag="rb", name="rb")
            nc.gpsimd.partition_broadcast(rb, recT)
            nc.vector.tensor_mul(
                attoutT_sb[:, h, c * 512 : (c + 1) * 512], avT, rb
            )

        pending = [None]  # block awaiting its normalization tail

        def attn_block(c, h):
            NK = 4 * c + 4
            step = 4
            probs = []
            for j in range(NK):
                r = j - 4 * c
                lo = P * r if r > 0 else 0
                sc = scp.tile([P, 512], f32, tag="sc", name="sc")
                nc.tensor.matmul(
                    sc[:, lo:512],
                    kT_sb[:, h, j * P : (j + 1) * P],
                    qT_sb[:, h, c * 512 + lo : (c + 1) * 512],
                    start=True,
                    stop=True,
                )
                pj = probp.tile([P, 512], bf16, tag="probs", name="pj")
                nc.scalar.activation(out=pj[:, lo:512], in_=sc[:, lo:512], func=Exp)
                if r >= 0:
                    # trimmed: no consumer ever reads pj[:, :lo]
                    nc.vector.tensor_mul(
                        pj[:, lo:512], pj[:, lo:512], bm_sb[:, r, lo:512]
                    )
                probs.append(pj)
                if j % step == step - 1:
                    filler()
                if j == 7 and pending[0] is not None:
                    # flush early: queues the aux's DVE work ahead of this
                    # block's later masks, shortening the chain's latency
                    emit_aux(pending[0])
                    pending[0] = None
            if pending[0] is not None:
                emit_aux(pending[0])
                pending[0] = None
            avT = avp.tile([P, 512], f32, tag="av", name="avT")
            for j in range(NK):
                r = j - 4 * c
                lo = P * r if r > 0 else 0
                nc.tensor.matmul(
                    avT[:, lo:512],
                    v_sb[:, j, h, :],
                    probs[j][:, lo:512],
                    start=(j == 0),
                    stop=(j == NK - 1),
                )
                for qt in range(max(r, 0), 4):
                    # all four chains share one PSUM zero region (the whole
                    # bank): only the first matmul starts it, only the last
                    # one stops it
                    nc.tensor.matmul(
                        scr[:, qt : qt + 1],
                        probs[j][:, qt * P : (qt + 1) * P],
                        ones_col,
                        start=(j == 0 and qt == max(r, 0)),
                        stop=(j == NK - 1 and qt == 3),
                    )
                if j % step == step - 1:
                    filler()
            rec = smallp.tile([P, 4], f32, tag="rec", name="rec")
            nc.vector.reciprocal(rec, scr[:, 0:4])
            pending[0] = (c, h, avT, rec)

        # ---- main wavefront ----
        # chunk 0 runs tile-major: four open PSUM chains consume each weight/x
        # tile as it lands, so the PE tracks the startup DMA stream
        for kind in ("q", "k", "v"):
            chains = [
                pp.tile([P, 512], f32, tag="pp", name="c0ps"),
                pp.tile([P, 512], f32, tag="pp", name="c0ps"),
                scp.tile([P, 512], f32, tag="sc", name="c0ps"),
                scp.tile([P, 512], f32, tag="sc", name="c0ps"),
            ]
            w_sb = {"q": wq_sb, "k": wk_sb, "v": wv_sb}[kind]
            for t in range(DK):
                for i in range(4):
                    if kind == "v":
                        nc.tensor.matmul(
                            chains[i],
                            xc0[:, t, i * P : (i + 1) * P],
                            wv_sb[:, t, :],
                            start=(t == 0),
                            stop=(t == DK - 1),
                        )
                    else:
                        nc.tensor.matmul(
                            chains[i],
                            w_sb[:, t, i * d : (i + 1) * d],
                            xc0[:, t, :],
                            start=(t == 0),
                            stop=(t == DK - 1),
                        )
            for i in range(4):
                if kind == "v":
                    nc.vector.tensor_copy(
                        v_sb[:, i, :, :],
                        chains[i].rearrange("p (h e) -> p h e", h=HPC),
                    )
                else:
                    dest = qT_sb if kind == "q" else kT_sb
                    nc.vector.tensor_copy(dest[:, i, 0:512], chains[i])

        deferred = []
        for c in range(CH):
            # queue fillers: next chunk's projections first
            if c + 1 < CH:
                if c + 2 < CH:
                    fifo.append(("dma", lambda cc=c + 2: dma_x_chunk(cc)))
                for kind, idx in _chunk_order():
                    fifo.append(
                        ("proj", lambda k=kind, i=idx, cc=c + 1: proj_group(cc, k, i))
                    )
            if c == CH - 1:
                # chunks deferred from row 2: ready filler for row 3's first
                # block, which otherwise has nothing to hide exp latency with
                fifo.extend(deferred)
                deferred = []
            for h in range(HPC):
                attn_block(c, h)
                if h == 0 and c >= 1:
                    # row c-1's attoutT is fully normalized once block (c, h0)
                    # has flushed the pending aux -> stage-3 row c-1 is safe
                    thunks = [
                        ("st3", lambda q=qt, n=nch: st3_chunk(q, n))
                        for qt in range(4 * (c - 1), 4 * c)
                        for nch in range(4)
                    ]
                    if c == CH - 2:
                        deferred = thunks[-8:]
                        thunks = thunks[:-8]
                    fifo.extend(thunks)
            if c == CH - 2:
                # drain only what row 3 depends on (chunk-3 projections and
                # x DMAs); leftover stage-3 chunks stay as row-3 fillers
                while fifo and fifo[0][0] != "st3":
                    fifo.pop(0)[1]()
            else:
                while fifo:
                    fifo.pop(0)[1]()

        # Final stage-3 row with a deep pipeline: borrow the (now idle)
        # scores banks, start the first 5 chunks with h0-h2 partial chains
        # so the h3 matmuls land after the last normalization, and alternate
        # copies DVE/ACT.
        STG = 6
        staged_pools = [
            (avp, "av"), (scp, "sc"), (scp, "sc"), (pp, "pp"), (scp, "sc"), (pp, "pp"),
        ]

        tail_pair = [None]

        def st3_tail(k, ps=None):
            qt, nch = 4 * (CH - 1) + k // 4, k % 4
            if ps is None:
                pool, tag = (scp, "sc") if k % 2 else (pp, "pp")
                ps = pool.tile([P, 512], f32, tag=tag, name="psy")
                h0 = 0
            else:
                h0 = HPC - 1
            for h in range(h0, HPC):
                nc.tensor.matmul(
                    ps,
                    attoutT_sb[:, h, qt * P : (qt + 1) * P],
                    wo_sb[:, h, nch * 512 : (nch + 1) * 512],
                    start=(h == 0),
                    stop=(h == HPC - 1),
                )
            # pair adjacent chunks into one [128,1024] staging tile and one
            # DMA: halves the HWDGE descriptor work that paces the tail
            if k % 2 == 0:
                yt2 = ysp.tile([P, 1024], bf16, tag="yt2", name="yt2", bufs=4)
                tail_pair[0] = yt2
                nc.scalar.copy(yt2[:, 0:512], ps)
            else:
                yt2 = tail_pair[0]
                nc.vector.tensor_copy(yt2[:, 512:1024], ps)
                nc.sync.dma_start(
                    out=y_r[:, qt, (nch - 1) * 512 : (nch + 1) * 512], in_=yt2
                )

        staged = []
        for k in range(STG):
            qt, nch = 4 * (CH - 1) + k // 4, k % 4
            pool, tag = staged_pools[k]
            ps = pool.tile([P, 512], f32, tag=tag, name="psy")
            for h in range(HPC - 1):
                nc.tensor.matmul(
                    ps,
                    attoutT_sb[:, h, qt * P : (qt + 1) * P],
                    wo_sb[:, h, nch * 512 : (nch + 1) * 512],
                    start=(h == 0),
                    stop=False,
                )
            staged.append(ps)
            if k == 0 and pending[0] is not None:
                emit_aux(pending[0], fast=True)
                pending[0] = None
        for k in range(STG):
            st3_tail(k, ps=staged[k])
        for k in range(STG, 16):
            st3_tail(k)

    nc.compile()
    return nc


def _chunk_order():
    # q-heads first (reuse wq while wk/wv stream in at startup), then k0 so
    # the row's first scores have kT, then v, then the remaining k-heads
    order = [("q", h) for h in range(HPC)]
    order += [("k", 0)]
    order += [("v", t) for t in range(4)]
    order += [("k", h) for h in range(1, HPC)]
    return order


def _static_inputs():
    import ml_dtypes

    masks = np.zeros((4, P, 512), dtype=np.float32)
    kk = np.arange(P)[:, None]
    qq = np.arange(512)[None, :]
    for r in range(4):
        masks[r] = (P * r + kk <= qq).astype(np.float32)
    return masks.astype(ml_dtypes.bfloat16), np.eye(P, dtype=np.float32)


def make_in_maps(x, Wq, Wk, Wv, Wo):
    """Shard full inputs into 8 per-core input dicts (bf16)."""
    import ml_dtypes

    bf = ml_dtypes.bfloat16
    bm, identb = _static_inputs()
    scale = 1.0 / math.sqrt(d)
    in_maps = []
    for c in range(N_CORES):
        b, g = divmod(c, 4)
        hs = g * HPC * d  # 512*g: rows of Wq for this head group
        in_maps.append(
            {
                "xT": np.ascontiguousarray(x[b].T).astype(bf),
                "wq": (np.ascontiguousarray(Wq[hs : hs + 512, :].T) * np.float32(scale)).astype(bf),
                "wk": np.ascontiguousarray(Wk[hs : hs + 512, :].T).astype(bf),
                "wv": np.ascontiguousarray(Wv[hs : hs + 512, :].T).astype(bf),
                "wo": np.ascontiguousarray(Wo[:, hs : hs + 512].T).astype(bf),
                "bm": bm,
                "identb": identb,
            }
        )
    return in_maps


def combine_results(results):
    """results: list of 8 dicts with 'y' [S, D] bf16 partials -> full [B, S, D]."""
    y = np.zeros((B, S, D), dtype=np.float32)
    for c in range(N_CORES):
        b = c // 4
        y[b] += np.asarray(results[c]["y"], dtype=np.float32)
    return y


def _is_canonical_causal(attn_mask):
    m = np.asarray(attn_mask).reshape(S, S)
    iu = np.triu_indices(S, k=1)
    if not np.all(m[iu] <= -1e8):
        return False
    il = np.tril_indices(S, k=0)
    return np.all(m[il] == 0.0)


def _scores_safe(x, Wq, Wk):
    """Sampled bound on |scores| to make exp-without-max safe."""
    rng = np.random.default_rng(0)
    qi = rng.choice(S, 96, replace=False)
    ki = rng.choice(S, 384, replace=False)
    mx = 0.0
    for b in range(B):
        q = (x[b][qi] @ Wq.T) / math.sqrt(d)  # [96, D]
        k = x[b][ki] @ Wk.T  # [384, D]
        qh = q.reshape(96, H, d)
        kh = k.reshape(384, H, d)
        s = np.einsum("qhd,khd->hqk", qh, kh)
        mx = max(mx, float(np.abs(s).max()))
    return mx < 30.0


def _numpy_reference(x, attn_mask, Wq, Wk, Wv, Wo):
    out = np.zeros((B, S, D), dtype=np.float32)
    m = np.asarray(attn_mask, dtype=np.float32).reshape(S, S)
    for b in range(B):
        q = (x[b] @ Wq.T).reshape(S, H, d).transpose(1, 0, 2)
        k = (x[b] @ Wk.T).reshape(S, H, d).transpose(1, 0, 2)
        v = (x[b] @ Wv.T).reshape(S, H, d).transpose(1, 0, 2)
        q = q / np.float32(math.sqrt(d))
        att_out = np.zeros((H, S, d), dtype=np.float32)
        for h in range(H):
            s = q[h] @ k[h].T + m
            s = s - s.max(axis=-1, keepdims=True)
            p = np.exp(s)
            p /= p.sum(axis=-1, keepdims=True)
            att_out[h] = p @ v[h]
        out[b] = att_out.transpose(1, 0, 2).reshape(S, D) @ Wo.T
    return out


def kernel(x, attn_mask, Wq, Wk, Wv, Wo):
    x = np.asarray(x, dtype=np.float32)
    Wq = np.asarray(Wq, dtype=np.float32)
    Wk = np.asarray(Wk, dtype=np.float32)
    Wv = np.asarray(Wv, dtype=np.float32)
    Wo = np.asarray(Wo, dtype=np.float32)

    if not _is_canonical_causal(attn_mask) or not _scores_safe(x, Wq, Wk):
        return _numpy_reference(x, attn_mask, Wq, Wk, Wv, Wo)

    from concourse.bass_utils import run_bass_kernel_spmd

    if "nc" not in _CACHE:
        _CACHE["nc"] = _build_module()
    nc = _CACHE["nc"]

    in_maps = make_in_maps(x, Wq, Wk, Wv, Wo)
    res = run_bass_kernel_spmd(nc, in_maps, core_ids=list(range(N_CORES)))
    return combine_results(res.results)


# revision 68
# speedup vs baseline: 1.2783x; 1.0140x over previous
"""Causal self-attention (B=2, S=2048, D=2048, H=16) on 8 TRN2 NeuronCores.

Sharding: tensor-parallel over heads x data-parallel over batch.
Core c = b*4 + g handles batch b and heads 4g..4g+3 (head_dim=128).

Single-NEFF SPMD design (all-bf16 matmuls, wavefront schedule):
  - x is streamed from DRAM once (bf16, transposed layout xT [D, S]), in 4
    chunks of 512 tokens.  Per chunk: q/k projections per head (qT/kT [d, S]
    bf16) and v projection (natural [tok, d] bf16), all bf16 matmuls with
    fp32 PSUM accumulation.
  - After chunk c, attention "row" c runs for all 4 heads: transposed scores
    sT[j] = k_j @ qT_block (f32 PSUM, trimmed to the causal triangle at
    128-col granularity), exp on ACT -> probs bf16, binary causal mask on
    diagonal-strip tiles (DVE), then avT[d, 512q] += v_j.T @ probs_j on PE.
    Softmax denominators come from near-free ap=1 matmuls
    (probs_j[:,qt].T @ ones).  Normalization: reciprocal (DVE), PE-transpose
    of the [128,4] rec vector, partition-broadcast (GPSIMD) to a [128,512]
    scale tile, one DVE multiply into attoutT bf16.
  - Stage-3 output projection chunks (y[qt] = sum_h attoutT_h.T @ Wo_h) are
    interleaved as PE "filler" work one row behind attention, so the ACT exp
    stream never stalls the PE.  y is written bf16; the host accumulates the
    4 per-core partials per batch in fp32.

Softmax skips the max-subtraction (scores are O(5..30) for the expected
input distribution; a host-side sampling guard falls back to a numpy
reference if scores could overflow exp, or if the mask is not the canonical
causal mask).
"""

import math
from contextlib import ExitStack

import numpy as np

B = 2
S = 2048
D = 2048
H = 16
HPC = 4  # heads per core
d = 128  # head dim
N_CORES = 8
P = 128
DK = D // P  # 16 contraction tiles
ST = S // P  # 16 token tiles
CH = S // 512  # 4 chunks of 512 tokens
PROBS_BUFS = 20

_CACHE = {}


def _build_module():
    import concourse.mybir as mybir
    import concourse.tile as tile
    from concourse import bacc

    f32 = mybir.dt.float32
    f32r = mybir.dt.float32r
    bf16 = mybir.dt.bfloat16
    Exp = mybir.ActivationFunctionType.Exp

    nc = bacc.Bacc("TRN2", target_bir_lowering=False, debug=False)

    xT = nc.dram_tensor("xT", [D, S], bf16, kind="ExternalInput")
    wq = nc.dram_tensor("wq", [D, HPC * d], bf16, kind="ExternalInput")
    wk = nc.dram_tensor("wk", [D, HPC * d], bf16, kind="ExternalInput")
    wv = nc.dram_tensor("wv", [D, HPC * d], bf16, kind="ExternalInput")
    wo = nc.dram_tensor("wo", [HPC * d, D], bf16, kind="ExternalInput")
    bm = nc.dram_tensor("bm", [4, P, 512], bf16, kind="ExternalInput")
    identb = nc.dram_tensor("identb", [P, P], f32, kind="ExternalInput")
    y = nc.dram_tensor("y", [S, D], bf16, kind="ExternalOutput")

    xT_r = xT.ap().rearrange("(t p) s -> p t s", p=P)
    wq_r = wq.ap().rearrange("(t p) m -> p t m", p=P)
    wk_r = wk.ap().rearrange("(t p) m -> p t m", p=P)
    wv_r = wv.ap().rearrange("(t p) m -> p t m", p=P)
    wo_r = wo.ap().rearrange("(t p) n -> p t n", p=P)
    y_r = y.ap().rearrange("(t p) n -> p t n", p=P)

    with tile.TileContext(nc) as tc, ExitStack() as top:
        consts = top.enter_context(tc.tile_pool(name="consts", bufs=1))
        wpool = top.enter_context(tc.tile_pool(name="wpool", bufs=1))
        wop = top.enter_context(tc.tile_pool(name="wop", bufs=1))
        xp = top.enter_context(tc.tile_pool(name="xp", bufs=2))
        qkp = top.enter_context(tc.tile_pool(name="qkp", bufs=1))
        vp = top.enter_context(tc.tile_pool(name="vp", bufs=1))
        aop = top.enter_context(tc.tile_pool(name="aop", bufs=1))
        probp = top.enter_context(tc.tile_pool(name="probp", bufs=PROBS_BUFS))
        smallp = top.enter_context(tc.tile_pool(name="smallp", bufs=3))
        ysp = top.enter_context(tc.tile_pool(name="ysp", bufs=6))
        # PSUM: pp(2) + sc(3) + av(2) + scr(1) = 8 banks
        pp = top.enter_context(tc.tile_pool(name="pp", bufs=2, space="PSUM"))
        scp = top.enter_context(tc.tile_pool(name="scp", bufs=3, space="PSUM"))
        avp = top.enter_context(tc.tile_pool(name="avp", bufs=2, space="PSUM"))
        scrp = top.enter_context(tc.tile_pool(name="scrp", bufs=1, space="PSUM"))

        bm_sb = consts.tile([P, 4, 512], bf16, tag="bm")
        id_sb = consts.tile([P, P], f32, tag="identb")
        ones_col = consts.tile([P, 1], bf16, tag="ones")
        wq_sb = wpool.tile([P, DK, HPC * d], bf16, tag="wq")
        wk_sb = wpool.tile([P, DK, HPC * d], bf16, tag="wk")
        wv_sb = wpool.tile([P, DK, HPC * d], bf16, tag="wv")
        wo_sb = wop.tile([P, HPC, D], bf16, tag="wo")
        qT_sb = qkp.tile([P, HPC, S], bf16, tag="qT")
        kT_sb = qkp.tile([P, HPC, S], bf16, tag="kT")
        v_sb = vp.tile([P, ST, HPC, d], bf16, tag="v")
        attoutT_sb = aop.tile([P, HPC, S], bf16, tag="attoutT")
        scr = scrp.tile([P, 512], f32, tag="scr")  # softmax denominators, cols 0:4

        nc.gpsimd.memset(ones_col, 1.0)
        warm = consts.tile([P, 512], bf16, tag="warm")
        nc.gpsimd.memset(warm, 0.0)
        wps = scp.tile([P, 512], f32, tag="sc", name="warmps")
        for wi in range(8):
            nc.tensor.matmul(
                warm if False else wps,
                warm[:, 0:P],
                warm,
                start=(wi == 0),
                stop=(wi == 7),
            )

        # ---- initial DMAs (SP ring, priority order) ----
        xc_tiles = [None] * CH

        def dma_x_chunk(c):
            t = xp.tile([P, DK, 512], bf16, tag="xc", name=f"xc{c}")
            for p4 in range(4):
                nc.sync.dma_start(
                    out=t[:, 4 * p4 : 4 * p4 + 4, :],
                    in_=xT_r[:, 4 * p4 : 4 * p4 + 4, c * 512 : (c + 1) * 512],
                )
            xc_tiles[c] = t

        xc0 = xp.tile([P, DK, 512], bf16, tag="xc", name="xc0")
        xc_tiles[0] = xc0
        # interleave wq/x0 in small pieces: the tile-major chunk-0 schedule
        # below consumes the stream at line rate
        bounds = [0, 1, 2, 4, 6, 8, 10, 12, 14, 16]
        for a, b in zip(bounds[:-1], bounds[1:]):
            nc.sync.dma_start(out=wq_sb[:, a:b, :], in_=wq_r[:, a:b, :])
            nc.sync.dma_start(out=xc0[:, a:b, :], in_=xT_r[:, a:b, 0:512])
        for w_sb, w_r in ((wk_sb, wk_r), (wv_sb, wv_r)):
            for p2 in range(8):
                nc.sync.dma_start(
                    out=w_sb[:, 2 * p2 : 2 * p2 + 2, :],
                    in_=w_r[:, 2 * p2 : 2 * p2 + 2, :],
                )
        dma_x_chunk(1)
        nc.sync.dma_start(out=bm_sb, in_=bm.ap().rearrange("r p m -> p r m"))
        nc.sync.dma_start(out=id_sb, in_=identb.ap())
        for p4 in range(4):
            nc.sync.dma_start(out=wo_sb[:, p4, :], in_=wo_r[:, p4, :])

        # ---- work units ----
        def proj_group(c, kind, idx):
            """kind: 'q'/'k' with idx=head, 'v' with idx=token-subtile."""
            xc = xc_tiles[c]
            ps = pp.tile([P, 512], f32, tag="pp", name="psp")
            if kind == "v":
                for kk in range(DK):
                    nc.tensor.matmul(
                        ps,
                        xc[:, kk, idx * P : (idx + 1) * P],
                        wv_sb[:, kk, :],
                        start=(kk == 0),
                        stop=(kk == DK - 1),
                    )
                nc.vector.tensor_copy(
                    v_sb[:, c * 4 + idx, :, :],
                    ps.rearrange("p (h e) -> p h e", h=HPC),
                )
            else:
                w_sb = wq_sb if kind == "q" else wk_sb
                dest = qT_sb if kind == "q" else kT_sb
                for kk in range(DK):
                    nc.tensor.matmul(
                        ps,
                        w_sb[:, kk, idx * d : (idx + 1) * d],
                        xc[:, kk, :],
                        start=(kk == 0),
                        stop=(kk == DK - 1),
                    )
                nc.vector.tensor_copy(dest[:, idx, c * 512 : (c + 1) * 512], ps)

        st3_state = {"mode": "dve", "alt": 0}

        def st3_chunk(qt, nch):
            ps = pp.tile([P, 512], f32, tag="pp", name="psy")
            for h in range(HPC):
                nc.tensor.matmul(
                    ps,
                    attoutT_sb[:, h, qt * P : (qt + 1) * P],
                    wo_sb[:, h, nch * 512 : (nch + 1) * 512],
                    start=(h == 0),
                    stop=(h == HPC - 1),
                )
            yt = ysp.tile([P, 512], bf16, tag="yt", name="yt")
            st3_state["alt"] += 1
            if st3_state["mode"] == "mix" and st3_state["alt"] % 2 == 0:
                nc.scalar.copy(yt, ps)
            else:
                nc.vector.tensor_copy(yt, ps)
            nc.sync.dma_start(out=y_r[:, qt, nch * 512 : (nch + 1) * 512], in_=yt)

        # filler fifo of (kind, thunk)
        fifo = []

        def filler(n=1):
            for _ in range(n):
                if fifo:
                    fifo.pop(0)[1]()

        def emit_aux(blk, fast=False):
            """Normalization tail of a finished block: one PE transpose of the
            [128,4] rec vector, DVE copy to SBUF, a tiny SBUF->SBUF DMA that
            flattens [4,128] onto partition 0, partition-broadcast (GPSIMD) to
            [128,512], then one DVE multiply applies 1/den along the free (q)
            axis of avT."""
            c, h, avT, rec = blk
            recT_ps = scp.tile([P, 512], f32, tag="sc", name="recT_ps")
            recT = smallp.tile([1, 512], bf16, tag="recT", name="recT")
            if fast:
                # latency-optimized tail for the last block: four transposes
                # put rec flat on partition 0 directly (no DMA round-trip)
                for qt in range(4):
                    nc.tensor.matmul(
                        recT_ps[0:1, qt * P : (qt + 1) * P],
                        rec[:, qt : qt + 1],
                        id_sb,
                        is_transpose=True,
                        start=(qt == 0),
                        stop=(qt == 3),
                    )
                nc.vector.tensor_copy(recT, recT_ps[0:1, 0:512])
            else:
                nc.tensor.transpose(recT_ps[0:4, 0:P], rec, id_sb)
                recT4 = smallp.tile([4, P], bf16, tag="recT4", name="recT4")
                nc.vector.tensor_copy(recT4, recT_ps[0:4, 0:P])
                nc.scalar.dma_start(out=recT, in_=recT4)
            filler()
            rb = smallp.tile([P, 512], bf16, tag="rb", name="rb")
            nc.gpsimd.partition_broadcast(rb, recT)
            nc.vector.tensor_mul(
                attoutT_sb[:, h, c * 512 : (c + 1) * 512], avT, rb
            )

        pending = [None]  # block awaiting its normalization tail

        def attn_block(c, h):
            NK = 4 * c + 4
            # front blocks rarely stall (exp backlog is short); save filler
            # work for the row's last blocks where avT waits on exp latency
            step = 4 if c < 2 else (8 if h < 2 else (4 if h == 2 else 2))
            probs = []
            for j in range(NK):
                r = j - 4 * c
                lo = P * r if r > 0 else 0
                sc = scp.tile([P, 512], f32, tag="sc", name="sc")
                nc.tensor.matmul(
                    sc[:, lo:512],
                    kT_sb[:, h, j * P : (j + 1) * P],
                    qT_sb[:, h, c * 512 + lo : (c + 1) * 512],
                    start=True,
                    stop=True,
                )
                pj = probp.tile([P, 512], bf16, tag="probs", name="pj")
                nc.scalar.activation(out=pj[:, lo:512], in_=sc[:, lo:512], func=Exp)
                if r >= 0:
                    # trimmed: no consumer ever reads pj[:, :lo]
                    nc.vector.tensor_mul(
                        pj[:, lo:512], pj[:, lo:512], bm_sb[:, r, lo:512]
                    )
                probs.append(pj)
                if j % step == step - 1:
                    filler()
                if j == 7 and pending[0] is not None:
                    # flush early: queues the aux's DVE work ahead of this
                    # block's later masks, shortening the chain's latency
                    emit_aux(pending[0])
                    pending[0] = None
            if pending[0] is not None:
                emit_aux(pending[0])
                pending[0] = None
            avT = avp.tile([P, 512], f32, tag="av", name="avT")
            for j in range(NK):
                r = j - 4 * c
                lo = P * r if r > 0 else 0
                nc.tensor.matmul(
                    avT[:, lo:512],
                    v_sb[:, j, h, :],
                    probs[j][:, lo:512],
                    start=(j == 0),
                    stop=(j == NK - 1),
                )
                for qt in range(max(r, 0), 4):
                    # all four chains share one PSUM zero region (the whole
                    # bank): only the first matmul starts it, only the last
                    # one stops it
                    nc.tensor.matmul(
                        scr[:, qt : qt + 1],
                        probs[j][:, qt * P : (qt + 1) * P],
                        ones_col,
                        start=(j == 0 and qt == max(r, 0)),
                        stop=(j == NK - 1 and qt == 3),
                    )
                if j % step == step - 1:
                    filler()
            rec = smallp.tile([P, 4], f32, tag="rec", name="rec")
            nc.vector.reciprocal(rec, scr[:, 0:4])
            pending[0] = (c, h, avT, rec)

        # ---- main wavefront ----
        # chunk 0 runs tile-major: four open PSUM chains consume each weight/x
        # tile as it lands, so the PE tracks the startup DMA stream
        for kind in ("q", "k", "v"):
            chains = [
                pp.tile([P, 512], f32, tag="pp", name="c0ps"),
                pp.tile([P, 512], f32, tag="pp", name="c0ps"),
                scp.tile([P, 512], f32, tag="sc", name="c0ps"),
                scp.tile([P, 512], f32, tag="sc", name="c0ps"),
            ]
            w_sb = {"q": wq_sb, "k": wk_sb, "v": wv_sb}[kind]
            for t in range(DK):
                for i in range(4):
                    if kind == "v":
                        nc.tensor.matmul(
                            chains[i],
                            xc0[:, t, i * P : (i + 1) * P],
                            wv_sb[:, t, :],
                            start=(t == 0),
                            stop=(t == DK - 1),
                        )
                    else:
                        nc.tensor.matmul(
                            chains[i],
                            w_sb[:, t, i * d : (i + 1) * d],
                            xc0[:, t, :],
                            start=(t == 0),
                            stop=(t == DK - 1),
                        )
            for i in range(4):
                if kind == "v":
                    nc.vector.tensor_copy(
                        v_sb[:, i, :, :],
                        chains[i].rearrange("p (h e) -> p h e", h=HPC),
                    )
                else:
                    dest = qT_sb if kind == "q" else kT_sb
                    nc.vector.tensor_copy(dest[:, i, 0:512], chains[i])

        deferred = []
        for c in range(CH):
            # queue fillers: next chunk's projections first
            if c + 1 < CH:
                if c + 2 < CH:
                    fifo.append(("dma", lambda cc=c + 2: dma_x_chunk(cc)))
                for kind, idx in _chunk_order():
                    fifo.append(
                        ("proj", lambda k=kind, i=idx, cc=c + 1: proj_group(cc, k, i))
                    )
            if c == CH - 1:
                # chunks deferred from row 2: ready filler for row 3's first
                # block, which otherwise has nothing to hide exp latency with
                fifo.extend(deferred)
                deferred = []
            for h in range(HPC):
                attn_block(c, h)
                if h == 0 and c >= 1:
                    # row c-1's attoutT is fully normalized once block (c, h0)
                    # has flushed the pending aux -> stage-3 row c-1 is safe
                    thunks = [
                        ("st3", lambda q=qt, n=nch: st3_chunk(q, n))
                        for qt in range(4 * (c - 1), 4 * c)
                        for nch in range(4)
                    ]
                    if c == CH - 2:
                        deferred = thunks[-8:]
                        thunks = thunks[:-8]
                    fifo.extend(thunks)
            # drain only what the next row depends on (projections and x
            # DMAs); surplus stage-3 chunks carry forward as future fillers
            if c < CH - 1:
                rest = []
                for it in fifo:
                    if it[0] == "st3":
                        rest.append(it)
                    else:
                        it[1]()
                fifo[:] = rest
            else:
                while fifo:
                    fifo.pop(0)[1]()

        # Final stage-3 row with a deep pipeline: borrow the (now idle)
        # scores banks, start the first 5 chunks with h0-h2 partial chains
        # so the h3 matmuls land after the last normalization, and alternate
        # copies DVE/ACT.
        STG = 6
        staged_pools = [
            (avp, "av"), (scp, "sc"), (scp, "sc"), (pp, "pp"), (scp, "sc"), (pp, "pp"),
        ]

        tail_pair = [None]

        def st3_tail(k, ps=None):
            qt, nch = 4 * (CH - 1) + k // 4, k % 4
            if ps is None:
                pool, tag = (scp, "sc") if k % 2 else (pp, "pp")
                ps = pool.tile([P, 512], f32, tag=tag, name="psy")
                h0 = 0
            else:
                h0 = HPC - 1
            for h in range(h0, HPC):
                nc.tensor.matmul(
                    ps,
                    attoutT_sb[:, h, qt * P : (qt + 1) * P],
                    wo_sb[:, h, nch * 512 : (nch + 1) * 512],
                    start=(h == 0),
                    stop=(h == HPC - 1),
                )
            # pair adjacent chunks into one [128,1024] staging tile and one
            # DMA (halves HWDGE descriptor work), except the final two chunks
            # which go out as singles so the terminal transfer is small
            if k >= 14:
                yt = ysp.tile([P, 512], bf16, tag="yt", name="yt")
                if k == 15:
                    nc.scalar.copy(yt, ps)
                else:
                    nc.vector.tensor_copy(yt, ps)
                nc.sync.dma_start(
                    out=y_r[:, qt, nch * 512 : (nch + 1) * 512], in_=yt
                )
            elif k % 2 == 0:
                yt2 = ysp.tile([P, 1024], bf16, tag="yt2", name="yt2", bufs=4)
                tail_pair[0] = yt2
                nc.scalar.copy(yt2[:, 0:512], ps)
            else:
                yt2 = tail_pair[0]
                nc.vector.tensor_copy(yt2[:, 512:1024], ps)
                nc.sync.dma_start(
                    out=y_r[:, qt, (nch - 1) * 512 : (nch + 1) * 512], in_=yt2
                )

        staged = []
        for k in range(STG):
            qt, nch = 4 * (CH - 1) + k // 4, k % 4
            pool, tag = staged_pools[k]
            ps = pool.tile([P, 512], f32, tag=tag, name="psy")
            for h in range(HPC - 1):
                nc.tensor.matmul(
                    ps,
                    attoutT_sb[:, h, qt * P : (qt + 1) * P],
                    wo_sb[:, h, nch * 512 : (nch + 1) * 512],
                    start=(h == 0),
                    stop=False,
                )
            staged.append(ps)
            if k == 0 and pending[0] is not None:
                emit_aux(pending[0], fast=True)
                pending[0] = None
        for k in range(STG):
            st3_tail(k, ps=staged[k])
        for k in range(STG, 16):
            st3_tail(k)

    nc.compile()
    return nc


def _chunk_order():
    # q-heads first (reuse wq while wk/wv stream in at startup), then k0 so
    # the row's first scores have kT, then v, then the remaining k-heads
    order = [("q", h) for h in range(HPC)]
    order += [("k", 0)]
    order += [("v", t) for t in range(4)]
    order += [("k", h) for h in range(1, HPC)]
    return order


def _static_inputs():
    import ml_dtypes

    masks = np.zeros((4, P, 512), dtype=np.float32)
    kk = np.arange(P)[:, None]
    qq = np.arange(512)[None, :]
    for r in range(4):
        masks[r] = (P * r + kk <= qq).astype(np.float32)
    return masks.astype(ml_dtypes.bfloat16), np.eye(P, dtype=np.float32)


def make_in_maps(x, Wq, Wk, Wv, Wo):
    """Shard full inputs into 8 per-core input dicts (bf16)."""
    import ml_dtypes

    bf = ml_dtypes.bfloat16
    bm, identb = _static_inputs()
    scale = 1.0 / math.sqrt(d)
    in_maps = []
    for c in range(N_CORES):
        b, g = divmod(c, 4)
        hs = g * HPC * d  # 512*g: rows of Wq for this head group
        in_maps.append(
            {
                "xT": np.ascontiguousarray(x[b].T).astype(bf),
                "wq": (np.ascontiguousarray(Wq[hs : hs + 512, :].T) * np.float32(scale)).astype(bf),
                "wk": np.ascontiguousarray(Wk[hs : hs + 512, :].T).astype(bf),
                "wv": np.ascontiguousarray(Wv[hs : hs + 512, :].T).astype(bf),
                "wo": np.ascontiguousarray(Wo[:, hs : hs + 512].T).astype(bf),
                "bm": bm,
                "identb": identb,
            }
        )
    return in_maps


def combine_results(results):
    """results: list of 8 dicts with 'y' [S, D] bf16 partials -> full [B, S, D]."""
    y = np.zeros((B, S, D), dtype=np.float32)
    for c in range(N_CORES):
        b = c // 4
        y[b] += np.asarray(results[c]["y"], dtype=np.float32)
    return y


def _is_canonical_causal(attn_mask):
    m = np.asarray(attn_mask).reshape(S, S)
    iu = np.triu_indices(S, k=1)
    if not np.all(m[iu] <= -1e8):
        return False
    il = np.tril_indices(S, k=0)
    return np.all(m[il] == 0.0)


def _scores_safe(x, Wq, Wk):
    """Sampled bound on |scores| to make exp-without-max safe."""
    rng = np.random.default_rng(0)
    qi = rng.choice(S, 96, replace=False)
    ki = rng.choice(S, 384, replace=False)
    mx = 0.0
    for b in range(B):
        q = (x[b][qi] @ Wq.T) / math.sqrt(d)  # [96, D]
        k = x[b][ki] @ Wk.T  # [384, D]
        qh = q.reshape(96, H, d)
        kh = k.reshape(384, H, d)
        s = np.einsum("qhd,khd->hqk", qh, kh)
        mx = max(mx, float(np.abs(s).max()))
    return mx < 30.0


def _numpy_reference(x, attn_mask, Wq, Wk, Wv, Wo):
    out = np.zeros((B, S, D), dtype=np.float32)
    m = np.asarray(attn_mask, dtype=np.float32).reshape(S, S)
    for b in range(B):
        q = (x[b] @ Wq.T).reshape(S, H, d).transpose(1, 0, 2)
        k = (x[b] @ Wk.T).reshape(S, H, d).transpose(1, 0, 2)
        v = (x[b] @ Wv.T).reshape(S, H, d).transpose(1, 0, 2)
        q = q / np.float32(math.sqrt(d))
        att_out = np.zeros((H, S, d), dtype=np.float32)
        for h in range(H):
            s = q[h] @ k[h].T + m
            s = s - s.max(axis=-1, keepdims=True)
            p = np.exp(s)
            p /= p.sum(axis=-1, keepdims=True)
            att_out[h] = p @ v[h]
        out[b] = att_out.transpose(1, 0, 2).reshape(S, D) @ Wo.T
    return out


def kernel(x, attn_mask, Wq, Wk, Wv, Wo):
    x = np.asarray(x, dtype=np.float32)
    Wq = np.asarray(Wq, dtype=np.float32)
    Wk = np.asarray(Wk, dtype=np.float32)
    Wv = np.asarray(Wv, dtype=np.float32)
    Wo = np.asarray(Wo, dtype=np.float32)

    if not _is_canonical_causal(attn_mask) or not _scores_safe(x, Wq, Wk):
        return _numpy_reference(x, attn_mask, Wq, Wk, Wv, Wo)

    from concourse.bass_utils import run_bass_kernel_spmd

    if "nc" not in _CACHE:
        _CACHE["nc"] = _build_module()
    nc = _CACHE["nc"]

    in_maps = make_in_maps(x, Wq, Wk, Wv, Wo)
    res = run_bass_kernel_spmd(nc, in_maps, core_ids=list(range(N_CORES)))
    return combine_results(res.results)
